# revision 37
# baseline (speedup 1.0000x reference)
"""Trainium2 Bass kernel for nn_ContextKGEModel (self-attentive path pooling + FFN hinge loss).

Data-parallel over the 2048 ragged groups, 8 NeuronCores:
  - Host: assign 16 whole batch rows per core (load-balanced), first-fit-
    decreasing-pack each core's 256 groups into 128-row bins, and ship
    triple_emb in two fp8-e4m3 layouts (row-major bins with an appended
    group-mask block, and a transposed copy in supertiles of 4 bins).
    Weights are replicated and pre-transposed; W1 is host-scaled by 8 and
    W2 by 16 (scalings fold into the sigmoid scale). A +/-1 pair-selection
    matrix (bf16) encodes the hinge pairs.
  - Device (per core): PE is kept continuously busy (warm-up matmuls hold the
    p-state clock at max). xwT = W_sfa^T @ X^T per supertile and the per-bin
    Gram run as fp8 DoubleRow matmuls; the group-masked column max runs as a
    single fused tensor_tensor_reduce per bin on DVE (tanh is monotone so it
    commutes with max); tanh+exp are batched per supertile on ACT; attention
    one-hot*exp rows are built on Pool+DVE; the pooled vectors accumulate
    TRANSPOSED in PSUM (lhsT = x rows) along with a ones-row that yields the
    softmax denominators, so no on-chip transposes or normalization pass are
    needed -- relu is positively homogeneous, so the 1/denom scaling folds
    into the final sigmoid step. PSUM->SBUF copies of xwT spread across
    ACT/DVE/Pool. FFN + hinge loss run on-chip; host sums 8 partial losses.
"""

import os
import threading
from contextlib import ExitStack

import numpy as np
import ml_dtypes

import concourse.bass as bass
import concourse.tile as tile
from concourse import mybir
from concourse.vector_clock import ScopedClock
from concourse.bass_utils import run_bass_kernel_spmd
from concourse.masks import make_identity

bf16 = ml_dtypes.bfloat16
fp8 = ml_dtypes.float8_e5m2
fp8e4 = ml_dtypes.float8_e4m3

B, NEG, L, D = 128, 15, 32, 768
NPAIR_SET = 120                      # 240 hinge pairs split into 2 matmul sets
G = B * (NEG + 1)
GAMMA = 0.1
NCORES = 8
ROWS_PER_CORE = B // NCORES          # 16 batch rows / core
SLOTS = ROWS_PER_CORE * (NEG + 1)    # 256 group slots / core
BIN = 128
KC = D // 128                        # 6 contraction chunks
HC = (4 * D) // 128                  # 24 hidden chunks
NEG_MASK = -240.0

_compile_cache = {}
_compile_lock = threading.Lock()


def _patch_tile_drain():
    """This walrus build rejects >1 sem-wait on an instruction ("Too many sync
    wait commands"); split the TileContext tail-drain waits across SP nops."""
    if getattr(tile.TileContext, "_drain_patch_applied", False):
        return

    def _drain_and_barrier(self, tick_clock, wait_clock):
        probe = self.nc.sync.nop(nofuse=True, hint="drain_wait_split")
        wait_clock.add_sem_waits(probe.ins, ScopedClock({None: tick_clock.global_clock}))
        si = probe.ins.sync_info
        waits = list(si.on_wait) if si is not None and si.on_wait else []
        if len(waits) > 1:
            si.on_wait = waits[:1]
            for w in waits[1:]:
                extra = self.nc.sync.nop(nofuse=True, hint="drain_wait_split")
                esi = extra.ins.sync_info
                if esi is None:
                    extra.ins.sync_info = mybir.SyncInfo(on_wait=[w], on_update=[])
                else:
                    esi.on_wait = [w]
        self.nc.sync.drain()
        self.nc.all_engine_barrier()
        assert self.sems is not None
        popped = self.nc._tile_sem_poison_stack.pop()
        assert popped is self._sem_poison
        self.nc.clear_and_free_semaphores(list(self.sems.allocated().values()))
        self.nc.all_engine_barrier()

    tile.TileContext._drain_and_barrier = _drain_and_barrier
    tile.TileContext._drain_patch_applied = True


_MAX_WAITS = 1


def _split_waits(nc, maxw=_MAX_WAITS):
    """Hoist excess sync-waits onto NoOps inserted just before the
    instruction on the same engine (walrus build caps waits/instruction)."""
    n_split = 0
    for fn in nc.m.functions:
        for bb in fn.blocks:
            out = []
            for inst in bb.instructions:
                si = inst.sync_info
                waits = list(si.on_wait) if si is not None and si.on_wait else []
                if len(waits) > maxw:
                    keep = waits[:maxw]
                    rest = waits[maxw:]
                    for i in range(0, len(rest), maxw):
                        n_split += 1
                        nop = mybir.InstNoOp(
                            name=f"WSPLIT-{n_split}",
                            engine=inst.engine,
                            debug=inst.debug,
                            ins=[], outs=[],
                            sync_info=mybir.SyncInfo(
                                on_wait=rest[i:i + maxw], on_update=[]),
                        )
                        out.append(nop)
                    si.on_wait = keep
                out.append(inst)
            if n_split:
                bb.instructions[:] = out
    return n_split


# ---------------------------------------------------------------- host packing

def _pack(sizes_flat):
    """Balanced batch-row -> core assignment, then per-half (8 rows = 128
    slots) first-fit-decreasing bin packing so each bin-pair's groups live in
    one 128-slot window."""
    sizes = sizes_flat.reshape(B, NEG + 1)
    row_load = sizes.sum(1)
    order = np.argsort(-row_load, kind="stable")
    core_rows = [[] for _ in range(NCORES)]
    core_load = np.zeros(NCORES, np.int64)
    for b in order:
        cands = [c for c in range(NCORES) if len(core_rows[c]) < ROWS_PER_CORE]
        c = min(cands, key=lambda c: core_load[c])
        core_rows[c].append(int(b))
        core_load[c] += row_load[b]
    bins_all = []
    halves_all = []
    for c in range(NCORES):
        # split the 16 rows into two halves with balanced total load
        rows_sorted = sorted(core_rows[c], key=lambda b: -row_load[b])
        half_rows = [[], []]
        hl = [0, 0]
        for b in rows_sorted:
            h = 0 if (hl[0] <= hl[1] and len(half_rows[0]) < 8) or                      len(half_rows[1]) >= 8 else 1
            half_rows[h].append(b)
            hl[h] += row_load[b]
        ordered = half_rows[0] + half_rows[1]
        core_rows[c] = ordered
        bins_c = []
        half_sizes = []
        for h in range(2):
            groups = []
            for lh, b in enumerate(half_rows[h]):
                lb = h * 8 + lh
                for k in range(NEG + 1):
                    g = b * (NEG + 1) + k
                    groups.append((g, lb * (NEG + 1) + k, int(sizes_flat[g])))
            groups.sort(key=lambda t: -t[2])
            bins = []
            for g, slot, n in groups:
                for bn in bins:
                    if bn[0] + n <= BIN:
                        bn[1].append((g, slot, n, bn[0]))
                        bn[0] += n
                        break
                else:
                    bins.append([n, [(g, slot, n, 0)]])
            hb = [bn[1] for bn in bins]
            if len(hb) % 2:
                hb.append([])
            half_sizes.append(len(hb))
            bins_c.extend(hb)
        bins_all.append(bins_c)
        halves_all.append(half_sizes[0])
    return core_rows, bins_all, halves_all


def _build_core_arrays(bins_c, triple_bf, offsets, NB):
    """Per-core packed device inputs (supertile-major layouts)."""
    NS = NB // 4
    X = np.zeros((NB, BIN, D), fp8e4)
    gid = np.full((NB, BIN), -1, np.int32)
    slot_of = np.full((NB, BIN), -1, np.int32)
    for bi, bn in enumerate(bins_c):
        for g, slot, n, off in bn:
            X[bi, off:off + n, :] = triple_bf[offsets[g]:offsets[g] + n].astype(fp8e4)
            gid[bi, off:off + n] = g
            slot_of[bi, off:off + n] = slot
    same = (gid[:, :, None] == gid[:, None, :]) & (gid[:, :, None] >= 0)
    m_add = np.where(same, np.float32(0.0), np.float32(NEG_MASK)).astype(fp8e4)
    # supertile-major packings
    xm = np.ascontiguousarray(
        X.reshape(NS, 4, BIN, D).transpose(0, 2, 1, 3).reshape(NS, BIN, 4 * D))
    madd_st = np.ascontiguousarray(
        m_add.reshape(NS, 4, BIN, BIN).transpose(0, 2, 1, 3)
             .reshape(NS, BIN, 4 * BIN))
    xt = np.ascontiguousarray(
        X.reshape(NS, 4, BIN, KC, 128)             # [s, b4, r, c, d]
         .transpose(0, 4, 3, 1, 2)                 # [s, d, c, b4, r]
         .reshape(NS, 128, KC, 4 * BIN))
    slot_st = np.ascontiguousarray(
        slot_of.astype(np.float32).reshape(NS, 4, BIN).transpose(2, 0, 1))  # [BIN,NS,4]
    return xm, madd_st, xt, slot_st


# ---------------------------------------------------------------- device program

N_WARM = 3          # PE warm-up matmuls (p-state hold through DMA startup)
WARM_F = 384        # free size of each warm-up matmul


def _build_program(NB, NP0, with_b1):
    NS = NB // 4
    NP = NB // 2
    nc = bass.Bass()
    dt = mybir.dt
    AF = mybir.ActivationFunctionType
    ALU = mybir.AluOpType

    x_d = nc.dram_tensor("x_bins", [NS, BIN, 4 * D], dt.float8e4, kind="ExternalInput")
    madd_d = nc.dram_tensor("madd_bins", [NS, BIN, 4 * BIN], dt.float8e4,
                            kind="ExternalInput")
    xt_d = nc.dram_tensor("xt_bins", [NS, 128, KC, 4 * BIN], dt.float8e4, kind="ExternalInput")
    slot_d = nc.dram_tensor("slot_of", [BIN, NS, 4], dt.float32, kind="ExternalInput")
    wsfa_d = nc.dram_tensor("w_sfa_t", [128, KC * D], dt.float8e4, kind="ExternalInput")
    w1t_d = nc.dram_tensor("w1_t", [128, KC * 4 * D], dt.float8e4, kind="ExternalInput")
    w2t_d = nc.dram_tensor("w2_t", [128, HC * 16], dt.float8e4, kind="ExternalInput")
    if with_b1:
        b1_d = nc.dram_tensor("b1_r", [1, HC * 128], dt.bfloat16, kind="ExternalInput")
    b2_d = nc.dram_tensor("b2_r", [1, 1], dt.float32, kind="ExternalInput")
    loss_d = nc.dram_tensor("loss", [1, 1], dt.float32, kind="ExternalOutput")
    DEBUG = bool(int(os.environ.get("KGE_DEBUG", "0")))
    if DEBUG:
        dbg_scores_d = nc.dram_tensor("dbg_scores", [1, SLOTS], dt.float32,
                                      kind="ExternalOutput")
        dbg_denom_d = nc.dram_tensor("dbg_denom", [1, SLOTS], dt.float32,
                                     kind="ExternalOutput")
        dbg_pt_d = nc.dram_tensor("dbg_pt", [128, KC, SLOTS], dt.float8e4,
                                  kind="ExternalOutput")
        dbg_att_d = nc.dram_tensor("dbg_att", [128, 2, SLOTS], dt.float8e4,
                                   kind="ExternalOutput")
        dbg_hr_d = nc.dram_tensor("dbg_hr", [128, HC, SLOTS], dt.float8e4,
                                  kind="ExternalOutput")
        dbg_spre_d = nc.dram_tensor("dbg_spre", [1, SLOTS], dt.float32,
                                    kind="ExternalOutput")
        dbg_pss_d = nc.dram_tensor("dbg_pss", [16, SLOTS], dt.float32,
                                   kind="ExternalOutput")

    with tile.TileContext(nc) as tc, ExitStack() as ctx:
        consts = ctx.enter_context(tc.tile_pool(name="consts", bufs=1))
        xres = ctx.enter_context(tc.tile_pool(name="xres", bufs=1))
        attres = ctx.enter_context(tc.tile_pool(name="attres", bufs=1))
        xt_pool = ctx.enter_context(tc.tile_pool(name="xt", bufs=4))
        xwt_pool = ctx.enter_context(tc.tile_pool(name="xwt", bufs=4))
        small = ctx.enter_context(tc.tile_pool(name="small", bufs=12))
        gm_pool = ctx.enter_context(tc.tile_pool(name="gm", bufs=6))
        ffn_pool = ctx.enter_context(tc.tile_pool(name="ffn", bufs=1))

        # resident constants / scratch
        wsfa = consts.tile([128, KC, D], dt.float8e4)      # [d_in_chunk, kc, e]
        slot_all = consts.tile([128, NS, 4], dt.float32)
        iota_i = consts.tile([128, SLOTS], dt.int32)
        nc.gpsimd.iota(iota_i, pattern=[[1, SLOTS]], base=0, channel_multiplier=0)
        iota_f = consts.tile([128, SLOTS], dt.float32)
        nc.vector.tensor_copy(iota_f, iota_i)
        ones2 = consts.tile([128, 2, 16], dt.float8e4)
        nc.vector.memset(ones2, 1.0)

        x_tiles = [xres.tile([128, 4 * D], dt.float8e4, tag=f"x{s}", name=f"x{s}")
                   for s in range(NS)]
        madd_tiles = [xres.tile([128, 4, BIN], dt.float8e4, tag=f"m{s}", name=f"m{s}")
                      for s in range(NS)]
        att_pairs = [attres.tile([128, 2, SLOTS], dt.float8e4, tag=f"a{p}", name=f"a{p}")
                     for p in range(NP)]
        w1t = consts.tile([128, KC, 4 * D], dt.float8e4)
        w2t = consts.tile([128, HC, 16], dt.float8e4)
        if with_b1:
            b1s = consts.tile([1, HC * 128], dt.bfloat16)
        b2s = consts.tile([1, 1], dt.float32)

        # ---- phase A: xwT per supertile; per-bin attention weights one
        # supertile behind; transposed-pooled accumulation two supertiles
        # behind (keeps PE off the ACT/DVE softmax critical path)
        with tc.tile_pool(name="ps_pool", bufs=1, space="PSUM") as ps_pooled, \
             ExitStack() as actx:
            ps_xw = actx.enter_context(tc.tile_pool(name="ps_xw", bufs=3, space="PSUM"))
            ps_gm = actx.enter_context(tc.tile_pool(name="ps_gm", bufs=1, space="PSUM"))
            xt_tiles = {}
            xwt_tiles = {}

            def emit_load(s):
                xt_t = xt_pool.tile([128, KC, 4 * BIN], dt.float8e4, tag="xt",
                                    name=f"xt{s}")
                nc.sync.dma_start(out=xt_t, in_=xt_d[s])
                nc.sync.dma_start(out=x_tiles[s], in_=x_d[s])
                nc.sync.dma_start(
                    out=madd_tiles[s],
                    in_=madd_d[s].rearrange("p (j i) -> p j i", i=BIN))
                xt_tiles[s] = xt_t

            # startup order: wsfa first (first xw matmul needs it), then the
            # first supertile's tiles, then everything else round-robin
            nc.sync.dma_start(out=wsfa, in_=wsfa_d[:, :].rearrange("p (k e) -> p k e", k=KC))
            emit_load(0)
            nc.sync.dma_start(out=slot_all, in_=slot_d[:, :, :])
            emit_load(1)

            def emit_weight_loads(part):
                w1v = w1t_d[:, :].rearrange("p (k h) -> p k h", k=KC)
                if part == 0:
                    nc.sync.dma_start(out=w1t[:, 0:2, :], in_=w1v[:, 0:2, :])
                    nc.sync.dma_start(out=w2t, in_=w2t_d[:, :].rearrange("p (h r) -> p h r", r=16))
                elif part == 1:
                    nc.sync.dma_start(out=w1t[:, 2:4, :], in_=w1v[:, 2:4, :])
                    if with_b1:
                        nc.sync.dma_start(out=b1s, in_=b1_d[:, :])
                    nc.sync.dma_start(out=b2s, in_=b2_d[:, :])
                else:
                    nc.sync.dma_start(out=w1t[:, 4:6, :], in_=w1v[:, 4:6, :])

            # xwT copy engines per e-chunk: 4x ACT, 2x DVE (Pool can't read PSUM)
            def _copy_xwt(dst, src, e):
                if e in (1, 4):
                    nc.vector.tensor_copy(dst, src)
                else:
                    nc.scalar.copy(dst, src)

            def emit_xw_mm(s):
                xt_t = xt_tiles[s]
                xwt_t = xwt_pool.tile([128, KC, 4 * BIN], dt.float8e4, tag="xwt",
                                      name=f"xwt{s}")
                pss = []
                for e in range(KC):
                    ps = ps_xw.tile([128, 4 * BIN], dt.float32, tag="psxw",
                                    name=f"psxw{s}_{e}")
                    for k in range(0, KC, 2):
                        nc.tensor.matmul(
                            ps, wsfa[:, k:k + 2, e * 128:(e + 1) * 128],
                            xt_t[:, k:k + 2, :],
                            start=(k == 0), stop=(k == KC - 2),
                            perf_mode=mybir.MatmulPerfMode.DoubleRow)
                    pss.append(ps)
                xwt_tiles[s] = xwt_t
                return pss

            def emit_copy(s, pss, e):
                xwt_t = xwt_tiles[s]
                if e in (1, 4):
                    nc.vector.tensor_copy(xwt_t[:, e, :], pss[e])
                else:
                    nc.scalar.copy(xwt_t[:, e, :], pss[e])

            def emit_gram(s):
                xt_t, xwt_t = xt_tiles[s], xwt_tiles[s]
                ps_g4 = ps_gm.tile([128, 4, BIN], dt.float32, tag="psgm",
                                   name=f"psgm{s}")
                for b in range(4):
                    sl = slice(b * BIN, (b + 1) * BIN)
                    for e in range(0, KC, 2):
                        nc.tensor.matmul(ps_g4[:, b, :], xwt_t[:, e:e + 2, sl],
                                         xt_t[:, e:e + 2, sl],
                                         start=(e == 0), stop=(e == KC - 2),
                                         perf_mode=mybir.MatmulPerfMode.DoubleRow)
                gram_ps[s] = ps_g4

            def emit_mask(s):
                madd_t = madd_tiles[s]
                ps_g4 = gram_ps[s]
                colmax4 = small.tile([128, 4], dt.bfloat16, tag="colmax",
                                     name=f"colmax{s}")
                scratch = gm_pool.tile([128, 4, BIN], dt.bfloat16, tag="gmm",
                                       name=f"gmm{s}")
                nc.vector.tensor_add(scratch, ps_g4, madd_t)
                nc.vector.tensor_reduce(out=colmax4, in_=scratch,
                                        op=ALU.max, axis=mybir.AxisListType.X)
                colmax_t[s] = colmax4

            def emit_acts(s):
                th4 = small.tile([128, 4], dt.float32, tag="th4", name=f"th{s}")
                nc.scalar.activation(th4, colmax_t[s], AF.Tanh)
                ex4 = small.tile([128, 4], dt.float32, tag="ex4", name=f"ex{s}")
                nc.scalar.activation(ex4, th4, AF.Exp)
                ex_t[s] = ex4

            def emit_att(s):
                slot_t = slot_all[:, s, :]
                ex4 = ex_t[s]
                last = (s == NS - 1)
                for b in range(4):
                    pi = s * 2 + b // 2
                    j = b % 2
                    eng = nc.vector if (last and b % 2 == 1) else nc.gpsimd
                    eng.tensor_scalar(
                        out=att_pairs[pi][:, j, :], in0=iota_f,
                        scalar1=slot_t[:, b:b + 1],
                        scalar2=ex4[:, b:b + 1],
                        op0=ALU.is_equal, op1=ALU.mult)

            # transposed pooled accumulation: ps_pT_k[k//2][:, k%2, w] holds
            # sum over rows of x[row, k-chunk] * att[row, slot-window]; the
            # ones-matmul rows in ps_pTd yield the softmax denominators
            ps_pT01 = ps_pooled.tile([128, 2, SLOTS], dt.float32, tag="pspT01",
                                     name="pspT01")
            ps_pT23 = ps_pooled.tile([128, 2, SLOTS], dt.float32, tag="pspT23",
                                     name="pspT23")
            ps_pT45 = ps_pooled.tile([128, 2, SLOTS], dt.float32, tag="pspT45",
                                     name="pspT45")
            ps_pTd = ps_pooled.tile([16, SLOTS], dt.float32, tag="pspTd",
                                    name="pspTd")
            ps_pT_k = [ps_pT01, ps_pT23, ps_pT45]

            def pT(k):
                return ps_pT_k[k // 2][:, k % 2, :]

            def emit_pooled(s):
                xv = x_tiles[s][:, :4 * D].rearrange("p (b w) -> p b w", w=D)
                for bp in range(2):
                    pi = s * 2 + bp
                    att_t = att_pairs[pi]
                    w = 0 if pi < NP0 else 1
                    sl = slice(w * 128, (w + 1) * 128)
                    stop = pi in (NP0 - 1, NP - 1)
                    for k in range(KC):
                        # one start per PSUM bank (first-ever touch); every
                        # other region's first write lands on pending-zero
                        nc.tensor.matmul(
                            pT(k)[:, sl],
                            xv[:, 2 * bp:2 * bp + 2, k * 128:(k + 1) * 128],
                            att_t[:, :, sl],
                            start=(pi == 0 and k % 2 == 0),
                            stop=stop,
                            perf_mode=mybir.MatmulPerfMode.DoubleRow,
                            skip_group_check=True)
                    nc.tensor.matmul(
                        ps_pTd[:, sl], ones2, att_t[:, :, sl],
                        start=(pi == 0), stop=stop,
                        perf_mode=mybir.MatmulPerfMode.DoubleRow,
                        skip_group_check=True)

            gram_ps = {}
            colmax_t = {}
            ex_t = {}
            for s in range(NS):
                if s >= 1:
                    emit_gram(s - 1)
                pss = emit_xw_mm(s)
                if s >= 1:
                    emit_mask(s - 1)          # DVE: add+reduce, ready earliest
                emit_copy(s, pss, 0)          # ACT
                emit_copy(s, pss, 1)          # DVE
                emit_copy(s, pss, 2)          # ACT
                emit_copy(s, pss, 3)          # ACT
                if s >= 1:
                    emit_acts(s - 1)          # ACT: tanh+exp between copies
                emit_copy(s, pss, 4)          # DVE
                emit_copy(s, pss, 5)          # ACT
                if s >= 1:
                    emit_att(s - 1)           # Pool
                if s + 2 < NS:
                    emit_load(s + 2)
                if s in (2, 4, 6):
                    emit_weight_loads((s - 2) // 2)
                if s >= 4:
                    emit_pooled(s - 4)
            emit_gram(NS - 1)
            emit_mask(NS - 1)
            emit_acts(NS - 1)
            emit_att(NS - 1)
            for sp in range(NS - 4, NS):
                emit_pooled(sp)
            actx.close()  # release ps_xw / ps_gm banks for phase B

            # ---- phase B1: pooled^T (scaled by 1/8 to keep fp8 range) to
            # SBUF; the denominators to SBUF fp32
            pooledT = ffn_pool.tile([128, KC, SLOTS], dt.float8e4, tag="pooledT")
            for k in range(KC):
                src_ap = pT(k)
                if k % 2 == 0:
                    nc.scalar.activation(pooledT[:, k, :], src_ap,
                                         AF.Copy, scale=0.125)
                else:
                    nc.vector.tensor_scalar(
                        out=pooledT[:, k, :], in0=src_ap,
                        scalar1=0.125, scalar2=None, op0=ALU.mult)
            denom = ffn_pool.tile([1, SLOTS], dt.float32, tag="denom")
            nc.vector.tensor_copy(denom, ps_pTd[0:1, :])
            if DEBUG:
                nc.sync.dma_start(out=dbg_denom_d[:, :], in_=denom)
                nc.sync.dma_start(out=dbg_pt_d[:, :, :], in_=pooledT)
                nc.sync.dma_start(out=dbg_att_d[:, :, :], in_=att_pairs[0])
            rdenom = ffn_pool.tile([1, SLOTS], dt.float32, tag="rdenom")
            nc.vector.reciprocal(rdenom, denom)

        # ---- phase B2: FFN + hinge loss
        with (
            tc.tile_pool(name="ps_h", bufs=4, space="PSUM") as ps_h,
            tc.tile_pool(name="ps_sc", bufs=1, space="PSUM") as ps_sc,
        ):
            # h = relu(W1 @ pooledT [+ denom*b1]); W1 host-scaled 8, pooled
            # scaled 1/8 -> psum holds true pre-activation
            hrelu = ffn_pool.tile([128, HC, SLOTS], dt.float8e4, tag="hrelu")
            ps_s = ps_sc.tile([16, SLOTS], dt.float32, tag="ps_s", name="ps_s")
            for hc in range(HC):
                ps_hh = ps_h.tile([128, SLOTS], dt.float32, tag="psh",
                                  name=f"psh{hc}")
                for k in range(0, KC, 2):
                    nc.tensor.matmul(ps_hh,
                                     w1t[:, k:k + 2, hc * 128:(hc + 1) * 128],
                                     pooledT[:, k:k + 2, :],
                                     start=(k == 0),
                                     stop=(k == KC - 2 and not with_b1),
                                     perf_mode=mybir.MatmulPerfMode.DoubleRow)
                if with_b1:
                    # bias for unnormalized pooled: + denom[slot]*b1[h]
                    nc.tensor.matmul(ps_hh, b1s[:, hc * 128:(hc + 1) * 128],
                                     denom, start=False, stop=True)
                if hc % 2 == 0:
                    nc.scalar.activation(hrelu[:, hc, :], ps_hh, AF.Relu)
                else:
                    nc.vector.tensor_scalar(
                        out=hrelu[:, hc, :], in0=ps_hh, scalar1=0.0,
                        scalar2=None, op0=ALU.max)
                # W2 contraction (DoubleRow over hidden-chunk pairs),
                # interleaved so PE never stalls on the relu chain
                if hc % 2 == 1 and hc >= 3:
                    h2 = hc - 3
                    nc.tensor.matmul(
                        ps_s, w2t[:, h2:h2 + 2, :],
                        hrelu[:, h2:h2 + 2, :],
                        start=(h2 == 0), stop=False,
                        perf_mode=mybir.MatmulPerfMode.DoubleRow)
            for h2 in (HC - 2,):
                nc.tensor.matmul(
                    ps_s, w2t[:, h2:h2 + 2, :],
                    hrelu[:, h2:h2 + 2, :],
                    start=False, stop=(h2 == HC - 2),
                    perf_mode=mybir.MatmulPerfMode.DoubleRow)

            # scores = sigmoid(ps_s / (16*denom) + b2); W1*8/8 and W2*16
            # leave psum = 16 * denom * true_score_pre
            spre = ffn_pool.tile([1, SLOTS], dt.float32, tag="spre")
            nc.vector.tensor_tensor(out=spre, in0=ps_s[0:1, :], in1=rdenom,
                                    op=ALU.mult)
            if DEBUG:
                nc.sync.dma_start(out=dbg_hr_d[:, :, :], in_=hrelu)
                nc.sync.dma_start(out=dbg_spre_d[:, :], in_=spre)
                pss_sb = ffn_pool.tile([16, SLOTS], dt.float32, tag="pss_sb")
                nc.vector.tensor_copy(pss_sb, ps_s)
                nc.sync.dma_start(out=dbg_pss_d[:, :], in_=pss_sb)
            scores = ffn_pool.tile([1, SLOTS], dt.float32, tag="scores")
            nc.scalar.activation(scores, spre, AF.Sigmoid, bias=b2s,
                                 scale=0.0625)
            if DEBUG:
                scf = ffn_pool.tile([1, SLOTS], dt.float32, tag="scf")
                nc.vector.tensor_copy(scf, scores)
                nc.sync.dma_start(out=dbg_scores_d[:, :], in_=scf)

            # hinge: per-slot relu(s - p_own_row + gamma) via a stride-0
            # broadcast AP (positive slots contribute exactly gamma each;
            # the host subtracts that constant from the summed loss)
            sc_ap = scores[0:1, :]
            p_bcast = bass.AP(tensor=sc_ap.tensor, offset=sc_ap.offset,
                              ap=[[sc_ap.ap[0][0], 1], [16, ROWS_PER_CORE],
                                  [0, NEG + 1]])
            hdiff = ffn_pool.tile([1, ROWS_PER_CORE, NEG + 1], dt.float32,
                                  tag="hdiff")
            nc.vector.tensor_tensor(
                out=hdiff, in0=sc_ap.rearrange("p (a b) -> p a b", b=NEG + 1),
                in1=p_bcast, op=ALU.subtract)
            nc.vector.tensor_scalar(out=hdiff, in0=hdiff, scalar1=GAMMA,
                                    scalar2=0.0, op0=ALU.add, op1=ALU.max)
            lsum = ffn_pool.tile([1, 1], dt.float32, tag="lsum")
            nc.vector.tensor_reduce(out=lsum, in_=hdiff, op=ALU.add,
                                    axis=mybir.AxisListType.XY)
            nc.sync.dma_start(out=loss_d[:, :], in_=lsum)

    _split_waits(nc)
    return nc


# ---------------------------------------------------------------- entry point

def kernel(triple_emb, W_sfa, W1, b1, W2, b2, tri2path_size):
    _patch_tile_drain()
    triple_emb = np.asarray(triple_emb, np.float32)
    sizes_flat = np.asarray(tri2path_size, np.int32).reshape(-1).astype(np.int64)
    offsets = np.concatenate([[0], np.cumsum(sizes_flat)[:-1]])

    core_rows, bins_all, halves_all = _pack(sizes_flat)
    NB = max(len(b) for b in bins_all)
    NB = ((NB + 3) // 4) * 4
    NP0 = max(h for h in halves_all) // 2
    # all cores must share one program: normalize each core's half boundary
    # by padding half0 with empty bins up to 2*NP0
    for c in range(NCORES):
        h0 = halves_all[c]
        if h0 < 2 * NP0:
            bins_all[c] = (bins_all[c][:h0] + [[]] * (2 * NP0 - h0)
                           + bins_all[c][h0:])
    NB = max(max(len(b) for b in bins_all), NB)
    NB = ((NB + 3) // 4) * 4

    b1_np = np.asarray(b1, np.float32)
    with_b1 = bool(np.any(b1_np != 0.0))

    triple_bf = triple_emb.astype(bf16)
    wsfa_t = np.ascontiguousarray(
        np.asarray(W_sfa, np.float32).T.reshape(KC, 128, D).transpose(1, 0, 2)
        .reshape(128, KC * D)).astype(fp8e4)
    w1_t = np.ascontiguousarray(
        (np.asarray(W1, np.float32) * 8.0).T.reshape(KC, 128, 4 * D)
        .transpose(1, 0, 2).reshape(128, KC * 4 * D)).astype(fp8e4)
    w2_t = np.ascontiguousarray(
        np.repeat((np.asarray(W2, np.float32) * 16.0).reshape(HC, 128).T
                  [:, :, None], 16, axis=2).reshape(128, HC * 16)).astype(fp8e4)
    b1_r = b1_np.reshape(1, HC * 128).astype(bf16)
    b2_r = np.asarray(b2, np.float32).reshape(1, 1)
    in_maps = []
    for c in range(NCORES):
        xm, madd_st, xt, slot_st = _build_core_arrays(bins_all[c], triple_bf,
                                                      offsets, NB)
        m = {
            "x_bins": xm, "madd_bins": madd_st, "xt_bins": xt, "slot_of": slot_st,
            "w_sfa_t": wsfa_t, "w1_t": w1_t, "w2_t": w2_t,
            "b2_r": b2_r,
        }
        if with_b1:
            m["b1_r"] = b1_r
        in_maps.append(m)

    with _compile_lock:
        key = (NB, NP0, with_b1)
        nc = _compile_cache.get(key)
        if nc is None:
            nc = _build_program(NB, NP0, with_b1)
            _compile_cache[key] = nc

    res = run_bass_kernel_spmd(nc, in_maps, core_ids=list(range(NCORES)),
                               trace=bool(int(os.environ.get("KGE_TRACE", "0"))))
    total = np.float64(0.0)
    for r in res.results:
        total += np.float64(r["loss"][0, 0])
    total -= np.float64(NCORES * ROWS_PER_CORE * GAMMA)
    kernel.last_results = res
    return np.asarray(np.float32(total))


# revision 40
# speedup vs baseline: 1.0605x; 1.0605x over previous
"""Trainium2 Bass kernel for nn_ContextKGEModel (self-attentive path pooling + FFN hinge loss).

Data-parallel over the 2048 ragged groups, 8 NeuronCores:
  - Host: assign 16 whole batch rows per core (load-balanced), first-fit-
    decreasing-pack each core's 256 groups into 128-row bins, and ship
    triple_emb in two fp8-e4m3 layouts (row-major bins with an appended
    group-mask block, and a transposed copy in supertiles of 4 bins).
    Weights are replicated and pre-transposed; W1 is host-scaled by 8 and
    W2 by 16 (scalings fold into the sigmoid scale). A +/-1 pair-selection
    matrix (bf16) encodes the hinge pairs.
  - Device (per core): PE is kept continuously busy (warm-up matmuls hold the
    p-state clock at max). xwT = W_sfa^T @ X^T per supertile and the per-bin
    Gram run as fp8 DoubleRow matmuls; the group-masked column max runs as a
    single fused tensor_tensor_reduce per bin on DVE (tanh is monotone so it
    commutes with max); tanh+exp are batched per supertile on ACT; attention
    one-hot*exp rows are built on Pool+DVE; the pooled vectors accumulate
    TRANSPOSED in PSUM (lhsT = x rows) along with a ones-row that yields the
    softmax denominators, so no on-chip transposes or normalization pass are
    needed -- relu is positively homogeneous, so the 1/denom scaling folds
    into the final sigmoid step. PSUM->SBUF copies of xwT spread across
    ACT/DVE/Pool. FFN + hinge loss run on-chip; host sums 8 partial losses.
"""

import os
import threading
from contextlib import ExitStack

import numpy as np
import ml_dtypes

import concourse.bass as bass
import concourse.tile as tile
from concourse import mybir
from concourse.vector_clock import ScopedClock
from concourse.bass_utils import run_bass_kernel_spmd
from concourse.masks import make_identity

bf16 = ml_dtypes.bfloat16
fp8 = ml_dtypes.float8_e5m2
fp8e4 = ml_dtypes.float8_e4m3

B, NEG, L, D = 128, 15, 32, 768
NPAIR_SET = 120                      # 240 hinge pairs split into 2 matmul sets
G = B * (NEG + 1)
GAMMA = 0.1
NCORES = 8
ROWS_PER_CORE = B // NCORES          # 16 batch rows / core
SLOTS = ROWS_PER_CORE * (NEG + 1)    # 256 group slots / core
BIN = 128
KC = D // 128                        # 6 contraction chunks
HC = (4 * D) // 128                  # 24 hidden chunks
NEG_MASK = -240.0

_compile_cache = {}
_compile_lock = threading.Lock()


def _patch_tile_drain():
    """This walrus build rejects >1 sem-wait on an instruction ("Too many sync
    wait commands"); split the TileContext tail-drain waits across SP nops."""
    if getattr(tile.TileContext, "_drain_patch_applied", False):
        return

    def _drain_and_barrier(self, tick_clock, wait_clock):
        probe = self.nc.sync.nop(nofuse=True, hint="drain_wait_split")
        wait_clock.add_sem_waits(probe.ins, ScopedClock({None: tick_clock.global_clock}))
        si = probe.ins.sync_info
        waits = list(si.on_wait) if si is not None and si.on_wait else []
        if len(waits) > 1:
            si.on_wait = waits[:1]
            for w in waits[1:]:
                extra = self.nc.sync.nop(nofuse=True, hint="drain_wait_split")
                esi = extra.ins.sync_info
                if esi is None:
                    extra.ins.sync_info = mybir.SyncInfo(on_wait=[w], on_update=[])
                else:
                    esi.on_wait = [w]
        self.nc.sync.drain()
        self.nc.all_engine_barrier()
        assert self.sems is not None
        popped = self.nc._tile_sem_poison_stack.pop()
        assert popped is self._sem_poison
        self.nc.clear_and_free_semaphores(list(self.sems.allocated().values()))
        self.nc.all_engine_barrier()

    tile.TileContext._drain_and_barrier = _drain_and_barrier
    tile.TileContext._drain_patch_applied = True


_MAX_WAITS = 1


def _split_waits(nc, maxw=_MAX_WAITS):
    """Hoist excess sync-waits onto NoOps inserted just before the
    instruction on the same engine (walrus build caps waits/instruction)."""
    n_split = 0
    for fn in nc.m.functions:
        for bb in fn.blocks:
            out = []
            for inst in bb.instructions:
                si = inst.sync_info
                waits = list(si.on_wait) if si is not None and si.on_wait else []
                if len(waits) > maxw:
                    keep = waits[:maxw]
                    rest = waits[maxw:]
                    for i in range(0, len(rest), maxw):
                        n_split += 1
                        nop = mybir.InstNoOp(
                            name=f"WSPLIT-{n_split}",
                            engine=inst.engine,
                            debug=inst.debug,
                            ins=[], outs=[],
                            sync_info=mybir.SyncInfo(
                                on_wait=rest[i:i + maxw], on_update=[]),
                        )
                        out.append(nop)
                    si.on_wait = keep
                out.append(inst)
            if n_split:
                bb.instructions[:] = out
    return n_split


# ---------------------------------------------------------------- host packing

def _pack(sizes_flat):
    """Balanced batch-row -> core assignment, then per-half (8 rows = 128
    slots) first-fit-decreasing bin packing so each bin-pair's groups live in
    one 128-slot window."""
    sizes = sizes_flat.reshape(B, NEG + 1)
    row_load = sizes.sum(1)
    order = np.argsort(-row_load, kind="stable")
    core_rows = [[] for _ in range(NCORES)]
    core_load = np.zeros(NCORES, np.int64)
    for b in order:
        cands = [c for c in range(NCORES) if len(core_rows[c]) < ROWS_PER_CORE]
        c = min(cands, key=lambda c: core_load[c])
        core_rows[c].append(int(b))
        core_load[c] += row_load[b]
    bins_all = []
    halves_all = []
    for c in range(NCORES):
        # split the 16 rows into two halves with balanced total load
        rows_sorted = sorted(core_rows[c], key=lambda b: -row_load[b])
        half_rows = [[], []]
        hl = [0, 0]
        for b in rows_sorted:
            h = 0 if (hl[0] <= hl[1] and len(half_rows[0]) < 8) or                      len(half_rows[1]) >= 8 else 1
            half_rows[h].append(b)
            hl[h] += row_load[b]
        ordered = half_rows[0] + half_rows[1]
        core_rows[c] = ordered
        bins_c = []
        half_sizes = []
        for h in range(2):
            groups = []
            for lh, b in enumerate(half_rows[h]):
                lb = h * 8 + lh
                for k in range(NEG + 1):
                    g = b * (NEG + 1) + k
                    groups.append((g, lb * (NEG + 1) + k, int(sizes_flat[g])))
            groups.sort(key=lambda t: -t[2])
            bins = []
            for g, slot, n in groups:
                for bn in bins:
                    if bn[0] + n <= BIN:
                        bn[1].append((g, slot, n, bn[0]))
                        bn[0] += n
                        break
                else:
                    bins.append([n, [(g, slot, n, 0)]])
            hb = [bn[1] for bn in bins]
            if len(hb) % 2:
                hb.append([])
            half_sizes.append(len(hb))
            bins_c.extend(hb)
        bins_all.append(bins_c)
        halves_all.append(half_sizes[0])
    return core_rows, bins_all, halves_all


def _build_core_arrays(bins_c, triple_bf, offsets, NB):
    """Per-core packed device inputs (supertile-major layouts)."""
    NS = NB // 4
    X = np.zeros((NB, BIN, D), fp8e4)
    gid = np.full((NB, BIN), -1, np.int32)
    slot_of = np.full((NB, BIN), -1, np.int32)
    for bi, bn in enumerate(bins_c):
        for g, slot, n, off in bn:
            X[bi, off:off + n, :] = triple_bf[offsets[g]:offsets[g] + n].astype(fp8e4)
            gid[bi, off:off + n] = g
            slot_of[bi, off:off + n] = slot
    same = (gid[:, :, None] == gid[:, None, :]) & (gid[:, :, None] >= 0)
    m_add = np.where(same, np.float32(0.0), np.float32(NEG_MASK)).astype(fp8e4)
    # supertile-major packings; x + mask merged into one DMA per supertile
    x_st = X.reshape(NS, 4, BIN, D).transpose(0, 2, 1, 3).reshape(NS, BIN, 4 * D)
    madd_st = m_add.reshape(NS, 4, BIN, BIN).transpose(0, 2, 1, 3) \
                   .reshape(NS, BIN, 4 * BIN)
    xm = np.ascontiguousarray(np.concatenate([x_st, madd_st], axis=2))
    xt = np.ascontiguousarray(
        X.reshape(NS, 4, BIN, KC, 128)             # [s, b4, r, c, d]
         .transpose(0, 4, 3, 1, 2)                 # [s, d, c, b4, r]
         .reshape(NS, 128, KC, 4 * BIN))
    slot_st = np.ascontiguousarray(
        slot_of.astype(np.float32).reshape(NS, 4, BIN).transpose(2, 0, 1))  # [BIN,NS,4]
    return xm, xt, slot_st


# ---------------------------------------------------------------- device program

N_WARM = 3          # PE warm-up matmuls (p-state hold through DMA startup)
WARM_F = 384        # free size of each warm-up matmul


def _build_program(NB, NP0, with_b1):
    NS = NB // 4
    NP = NB // 2
    nc = bass.Bass()
    dt = mybir.dt
    AF = mybir.ActivationFunctionType
    ALU = mybir.AluOpType

    XMW = 4 * D + 4 * BIN  # x rows + mask columns, fp8 bytes per partition
    x_d = nc.dram_tensor("x_bins", [NS, BIN, XMW], dt.float8e4, kind="ExternalInput")
    xt_d = nc.dram_tensor("xt_bins", [NS, 128, KC, 4 * BIN], dt.float8e4, kind="ExternalInput")
    slot_d = nc.dram_tensor("slot_of", [BIN, NS, 4], dt.float32, kind="ExternalInput")
    wsfa_d = nc.dram_tensor("w_sfa_t", [128, KC * D], dt.float8e4, kind="ExternalInput")
    w1t_d = nc.dram_tensor("w1_t", [128, KC * 4 * D], dt.float8e4, kind="ExternalInput")
    w2t_d = nc.dram_tensor("w2_t", [128, HC * 16], dt.float8e4, kind="ExternalInput")
    if with_b1:
        b1_d = nc.dram_tensor("b1_r", [1, HC * 128], dt.bfloat16, kind="ExternalInput")
    b2_d = nc.dram_tensor("b2_r", [1, 1], dt.float32, kind="ExternalInput")
    loss_d = nc.dram_tensor("loss", [1, 1], dt.float32, kind="ExternalOutput")
    DEBUG = bool(int(os.environ.get("KGE_DEBUG", "0")))
    if DEBUG:
        dbg_scores_d = nc.dram_tensor("dbg_scores", [1, SLOTS], dt.float32,
                                      kind="ExternalOutput")
        dbg_denom_d = nc.dram_tensor("dbg_denom", [1, SLOTS], dt.float32,
                                     kind="ExternalOutput")
        dbg_pt_d = nc.dram_tensor("dbg_pt", [128, KC, SLOTS], dt.float8e4,
                                  kind="ExternalOutput")
        dbg_att_d = nc.dram_tensor("dbg_att", [128, 2, SLOTS], dt.float8e4,
                                   kind="ExternalOutput")
        dbg_hr_d = nc.dram_tensor("dbg_hr", [128, HC, SLOTS], dt.float8e4,
                                  kind="ExternalOutput")
        dbg_spre_d = nc.dram_tensor("dbg_spre", [1, SLOTS], dt.float32,
                                    kind="ExternalOutput")
        dbg_pss_d = nc.dram_tensor("dbg_pss", [16, SLOTS], dt.float32,
                                   kind="ExternalOutput")

    with tile.TileContext(nc) as tc, ExitStack() as ctx:
        consts = ctx.enter_context(tc.tile_pool(name="consts", bufs=1))
        xres = ctx.enter_context(tc.tile_pool(name="xres", bufs=1))
        attres = ctx.enter_context(tc.tile_pool(name="attres", bufs=1))
        xt_pool = ctx.enter_context(tc.tile_pool(name="xt", bufs=4))
        xwt_pool = ctx.enter_context(tc.tile_pool(name="xwt", bufs=4))
        small = ctx.enter_context(tc.tile_pool(name="small", bufs=12))
        gm_pool = ctx.enter_context(tc.tile_pool(name="gm", bufs=6))
        ffn_pool = ctx.enter_context(tc.tile_pool(name="ffn", bufs=1))

        # resident constants / scratch
        wsfa = consts.tile([128, KC, D], dt.float8e4)      # [d_in_chunk, kc, e]
        slot_all = consts.tile([128, NS, 4], dt.float32)
        iota_i = consts.tile([128, SLOTS], dt.int32)
        nc.gpsimd.iota(iota_i, pattern=[[1, SLOTS]], base=0, channel_multiplier=0)
        iota_f = consts.tile([128, SLOTS], dt.float32)
        nc.vector.tensor_copy(iota_f, iota_i)
        ones2 = consts.tile([128, 2, 16], dt.float8e4)
        nc.vector.memset(ones2, 1.0)

        x_tiles = [xres.tile([128, XMW], dt.float8e4, tag=f"x{s}", name=f"x{s}")
                   for s in range(NS)]
        att_pairs = [attres.tile([128, 2, SLOTS], dt.float8e4, tag=f"a{p}", name=f"a{p}")
                     for p in range(NP)]
        w1t = consts.tile([128, KC, 4 * D], dt.float8e4)
        w2t = consts.tile([128, HC, 16], dt.float8e4)
        if with_b1:
            b1s = consts.tile([1, HC * 128], dt.bfloat16)
        b2s = consts.tile([1, 1], dt.float32)

        # ---- phase A: xwT per supertile; per-bin attention weights one
        # supertile behind; transposed-pooled accumulation two supertiles
        # behind (keeps PE off the ACT/DVE softmax critical path)
        with tc.tile_pool(name="ps_pool", bufs=1, space="PSUM") as ps_pooled, \
             ExitStack() as actx:
            ps_xw = actx.enter_context(tc.tile_pool(name="ps_xw", bufs=3, space="PSUM"))
            ps_gm = actx.enter_context(tc.tile_pool(name="ps_gm", bufs=1, space="PSUM"))
            xt_tiles = {}
            xwt_tiles = {}

            def emit_load(s):
                xt_t = xt_pool.tile([128, KC, 4 * BIN], dt.float8e4, tag="xt",
                                    name=f"xt{s}")
                nc.sync.dma_start(out=xt_t, in_=xt_d[s])
                nc.sync.dma_start(out=x_tiles[s], in_=x_d[s])
                xt_tiles[s] = xt_t

            # startup order: wsfa first (first xw matmul needs it), then the
            # first supertile's tiles, then everything else round-robin
            nc.sync.dma_start(out=wsfa, in_=wsfa_d[:, :].rearrange("p (k e) -> p k e", k=KC))
            emit_load(0)
            nc.sync.dma_start(out=slot_all, in_=slot_d[:, :, :])
            emit_load(1)

            def emit_weight_loads(part):
                w1v = w1t_d[:, :].rearrange("p (k h) -> p k h", k=KC)
                if part == 0:
                    nc.sync.dma_start(out=w1t[:, 0:2, :], in_=w1v[:, 0:2, :])
                    nc.sync.dma_start(out=w2t, in_=w2t_d[:, :].rearrange("p (h r) -> p h r", r=16))
                elif part == 1:
                    nc.sync.dma_start(out=w1t[:, 2:4, :], in_=w1v[:, 2:4, :])
                    if with_b1:
                        nc.sync.dma_start(out=b1s, in_=b1_d[:, :])
                    nc.sync.dma_start(out=b2s, in_=b2_d[:, :])
                else:
                    nc.sync.dma_start(out=w1t[:, 4:6, :], in_=w1v[:, 4:6, :])

            # xwT copy engines per e-chunk: 4x ACT, 2x DVE (Pool can't read PSUM)
            def _copy_xwt(dst, src, e):
                if e in (1, 4):
                    nc.vector.tensor_copy(dst, src)
                else:
                    nc.scalar.copy(dst, src)

            def emit_xw_mm(s):
                xt_t = xt_tiles[s]
                xwt_t = xwt_pool.tile([128, KC, 4 * BIN], dt.float8e4, tag="xwt",
                                      name=f"xwt{s}")
                pss = []
                for e in range(KC):
                    ps = ps_xw.tile([128, 4 * BIN], dt.float32, tag="psxw",
                                    name=f"psxw{s}_{e}")
                    for k in range(0, KC, 2):
                        nc.tensor.matmul(
                            ps, wsfa[:, k:k + 2, e * 128:(e + 1) * 128],
                            xt_t[:, k:k + 2, :],
                            start=(k == 0), stop=(k == KC - 2),
                            perf_mode=mybir.MatmulPerfMode.DoubleRow)
                    pss.append(ps)
                xwt_tiles[s] = xwt_t
                return pss

            def emit_copy(s, pss, e):
                xwt_t = xwt_tiles[s]
                if e in (1, 4):
                    nc.vector.tensor_copy(xwt_t[:, e, :], pss[e])
                else:
                    nc.scalar.copy(xwt_t[:, e, :], pss[e])

            def emit_gram(s):
                xt_t, xwt_t = xt_tiles[s], xwt_tiles[s]
                ps_g4 = ps_gm.tile([128, 4, BIN], dt.float32, tag="psgm",
                                   name=f"psgm{s}")
                for b in range(4):
                    sl = slice(b * BIN, (b + 1) * BIN)
                    for e in range(0, KC, 2):
                        nc.tensor.matmul(ps_g4[:, b, :], xwt_t[:, e:e + 2, sl],
                                         xt_t[:, e:e + 2, sl],
                                         start=(e == 0), stop=(e == KC - 2),
                                         perf_mode=mybir.MatmulPerfMode.DoubleRow)
                gram_ps[s] = ps_g4

            def emit_mask(s):
                madd_t = x_tiles[s][:, 4 * D:].rearrange("p (j i) -> p j i", i=BIN)
                ps_g4 = gram_ps[s]
                colmax4 = small.tile([128, 4], dt.bfloat16, tag="colmax",
                                     name=f"colmax{s}")
                scratch = gm_pool.tile([128, 4, BIN], dt.bfloat16, tag="gmm",
                                       name=f"gmm{s}")
                nc.vector.tensor_add(scratch, ps_g4, madd_t)
                nc.vector.tensor_reduce(out=colmax4, in_=scratch,
                                        op=ALU.max, axis=mybir.AxisListType.X)
                colmax_t[s] = colmax4

            def emit_acts(s):
                th4 = small.tile([128, 4], dt.float32, tag="th4", name=f"th{s}")
                nc.scalar.activation(th4, colmax_t[s], AF.Tanh)
                ex4 = small.tile([128, 4], dt.float32, tag="ex4", name=f"ex{s}")
                nc.scalar.activation(ex4, th4, AF.Exp)
                ex_t[s] = ex4

            def emit_att(s):
                slot_t = slot_all[:, s, :]
                ex4 = ex_t[s]
                last = (s == NS - 1)
                for b in range(4):
                    pi = s * 2 + b // 2
                    j = b % 2
                    eng = nc.vector if (last and b % 2 == 1) else nc.gpsimd
                    eng.tensor_scalar(
                        out=att_pairs[pi][:, j, :], in0=iota_f,
                        scalar1=slot_t[:, b:b + 1],
                        scalar2=ex4[:, b:b + 1],
                        op0=ALU.is_equal, op1=ALU.mult)

            # transposed pooled accumulation: ps_pT_k[k//2][:, k%2, w] holds
            # sum over rows of x[row, k-chunk] * att[row, slot-window]; the
            # ones-matmul rows in ps_pTd yield the softmax denominators
            ps_pT01 = ps_pooled.tile([128, 2, SLOTS], dt.float32, tag="pspT01",
                                     name="pspT01")
            ps_pT23 = ps_pooled.tile([128, 2, SLOTS], dt.float32, tag="pspT23",
                                     name="pspT23")
            ps_pT45 = ps_pooled.tile([128, 2, SLOTS], dt.float32, tag="pspT45",
                                     name="pspT45")
            ps_pTd = ps_pooled.tile([16, SLOTS], dt.float32, tag="pspTd",
                                    name="pspTd")
            ps_pT_k = [ps_pT01, ps_pT23, ps_pT45]

            def pT(k):
                return ps_pT_k[k // 2][:, k % 2, :]

            def emit_pooled(s):
                xv = x_tiles[s][:, :4 * D].rearrange("p (b w) -> p b w", w=D)
                for bp in range(2):
                    pi = s * 2 + bp
                    att_t = att_pairs[pi]
                    w = 0 if pi < NP0 else 1
                    sl = slice(w * 128, (w + 1) * 128)
                    stop = pi in (NP0 - 1, NP - 1)
                    for k in range(KC):
                        # one start per PSUM bank (first-ever touch); every
                        # other region's first write lands on pending-zero
                        nc.tensor.matmul(
                            pT(k)[:, sl],
                            xv[:, 2 * bp:2 * bp + 2, k * 128:(k + 1) * 128],
                            att_t[:, :, sl],
                            start=(pi == 0 and k % 2 == 0),
                            stop=stop,
                            perf_mode=mybir.MatmulPerfMode.DoubleRow,
                            skip_group_check=True)
                    nc.tensor.matmul(
                        ps_pTd[:, sl], ones2, att_t[:, :, sl],
                        start=(pi == 0), stop=stop,
                        perf_mode=mybir.MatmulPerfMode.DoubleRow,
                        skip_group_check=True)

            gram_ps = {}
            colmax_t = {}
            ex_t = {}
            for s in range(NS):
                pss = emit_xw_mm(s)
                if s >= 1:
                    emit_gram(s - 1)          # PE: after xw so copies of s-1
                                              # are long done
                emit_copy(s, pss, 1)          # DVE (c1 first: bank for e4)
                emit_copy(s, pss, 0)          # ACT
                emit_copy(s, pss, 2)          # ACT
                emit_copy(s, pss, 4)          # DVE (bank for next e1)
                emit_copy(s, pss, 3)          # ACT
                emit_copy(s, pss, 5)          # ACT
                if s >= 1:
                    emit_mask(s - 1)          # DVE add+reduce after c1/c4
                if s >= 1:
                    emit_acts(s - 1)          # ACT tanh+exp after c5
                if s >= 1:
                    emit_att(s - 1)           # Pool
                if s + 2 < NS:
                    emit_load(s + 2)
                if s in (2, 4, 6):
                    emit_weight_loads((s - 2) // 2)
                if s >= 4:
                    emit_pooled(s - 4)
            emit_gram(NS - 1)
            emit_mask(NS - 1)
            emit_acts(NS - 1)
            emit_att(NS - 1)
            for sp in range(NS - 4, NS):
                emit_pooled(sp)
            actx.close()  # release ps_xw / ps_gm banks for phase B

            # ---- phase B1: pooled^T (scaled by 1/8 to keep fp8 range) to
            # SBUF; the denominators to SBUF fp32
            pooledT = ffn_pool.tile([128, KC, SLOTS], dt.float8e4, tag="pooledT")
            for k in range(KC):
                src_ap = pT(k)
                if k % 2 == 0:
                    nc.scalar.activation(pooledT[:, k, :], src_ap,
                                         AF.Copy, scale=0.125)
                else:
                    nc.vector.tensor_scalar(
                        out=pooledT[:, k, :], in0=src_ap,
                        scalar1=0.125, scalar2=None, op0=ALU.mult)
            denom = ffn_pool.tile([1, SLOTS], dt.float32, tag="denom")
            nc.vector.tensor_copy(denom, ps_pTd[0:1, :])
            if DEBUG:
                nc.sync.dma_start(out=dbg_denom_d[:, :], in_=denom)
                nc.sync.dma_start(out=dbg_pt_d[:, :, :], in_=pooledT)
                nc.sync.dma_start(out=dbg_att_d[:, :, :], in_=att_pairs[0])
            rdenom = ffn_pool.tile([1, SLOTS], dt.float32, tag="rdenom")
            nc.vector.reciprocal(rdenom, denom)

        # ---- phase B2: FFN + hinge loss
        with (
            tc.tile_pool(name="ps_h", bufs=4, space="PSUM") as ps_h,
            tc.tile_pool(name="ps_sc", bufs=1, space="PSUM") as ps_sc,
        ):
            # h = relu(W1 @ pooledT [+ denom*b1]); W1 host-scaled 8, pooled
            # scaled 1/8 -> psum holds true pre-activation
            hrelu = ffn_pool.tile([128, HC, SLOTS], dt.float8e4, tag="hrelu")
            ps_s = ps_sc.tile([16, SLOTS], dt.float32, tag="ps_s", name="ps_s")
            for hc in range(HC):
                ps_hh = ps_h.tile([128, SLOTS], dt.float32, tag="psh",
                                  name=f"psh{hc}")
                for k in range(0, KC, 2):
                    nc.tensor.matmul(ps_hh,
                                     w1t[:, k:k + 2, hc * 128:(hc + 1) * 128],
                                     pooledT[:, k:k + 2, :],
                                     start=(k == 0),
                                     stop=(k == KC - 2 and not with_b1),
                                     perf_mode=mybir.MatmulPerfMode.DoubleRow)
                if with_b1:
                    # bias for unnormalized pooled: + denom[slot]*b1[h]
                    nc.tensor.matmul(ps_hh, b1s[:, hc * 128:(hc + 1) * 128],
                                     denom, start=False, stop=True)
                if hc % 2 == 0:
                    nc.scalar.activation(hrelu[:, hc, :], ps_hh, AF.Relu)
                else:
                    nc.vector.tensor_scalar(
                        out=hrelu[:, hc, :], in0=ps_hh, scalar1=0.0,
                        scalar2=None, op0=ALU.max)
                # W2 contraction (DoubleRow over hidden-chunk pairs),
                # interleaved so PE never stalls on the relu chain
                if hc % 2 == 1 and hc >= 3:
                    h2 = hc - 3
                    nc.tensor.matmul(
                        ps_s, w2t[:, h2:h2 + 2, :],
                        hrelu[:, h2:h2 + 2, :],
                        start=(h2 == 0), stop=False,
                        perf_mode=mybir.MatmulPerfMode.DoubleRow)
            for h2 in (HC - 2,):
                nc.tensor.matmul(
                    ps_s, w2t[:, h2:h2 + 2, :],
                    hrelu[:, h2:h2 + 2, :],
                    start=False, stop=(h2 == HC - 2),
                    perf_mode=mybir.MatmulPerfMode.DoubleRow)

            # scores = sigmoid(ps_s / (16*denom) + b2); W1*8/8 and W2*16
            # leave psum = 16 * denom * true_score_pre
            spre = ffn_pool.tile([1, SLOTS], dt.float32, tag="spre")
            nc.vector.tensor_tensor(out=spre, in0=ps_s[0:1, :], in1=rdenom,
                                    op=ALU.mult)
            if DEBUG:
                nc.sync.dma_start(out=dbg_hr_d[:, :, :], in_=hrelu)
                nc.sync.dma_start(out=dbg_spre_d[:, :], in_=spre)
                pss_sb = ffn_pool.tile([16, SLOTS], dt.float32, tag="pss_sb")
                nc.vector.tensor_copy(pss_sb, ps_s)
                nc.sync.dma_start(out=dbg_pss_d[:, :], in_=pss_sb)
            scores = ffn_pool.tile([1, SLOTS], dt.float32, tag="scores")
            nc.scalar.activation(scores, spre, AF.Sigmoid, bias=b2s,
                                 scale=0.0625)
            if DEBUG:
                scf = ffn_pool.tile([1, SLOTS], dt.float32, tag="scf")
                nc.vector.tensor_copy(scf, scores)
                nc.sync.dma_start(out=dbg_scores_d[:, :], in_=scf)

            # hinge: per-slot relu(s - p_own_row + gamma) via a stride-0
            # broadcast AP (positive slots contribute exactly gamma each;
            # the host subtracts that constant from the summed loss)
            sc_ap = scores[0:1, :]
            p_bcast = bass.AP(tensor=sc_ap.tensor, offset=sc_ap.offset,
                              ap=[[sc_ap.ap[0][0], 1], [16, ROWS_PER_CORE],
                                  [0, NEG + 1]])
            hdiff = ffn_pool.tile([1, ROWS_PER_CORE, NEG + 1], dt.float32,
                                  tag="hdiff")
            nc.vector.tensor_tensor(
                out=hdiff, in0=sc_ap.rearrange("p (a b) -> p a b", b=NEG + 1),
                in1=p_bcast, op=ALU.subtract)
            nc.vector.tensor_scalar(out=hdiff, in0=hdiff, scalar1=GAMMA,
                                    scalar2=0.0, op0=ALU.add, op1=ALU.max)
            lsum = ffn_pool.tile([1, 1], dt.float32, tag="lsum")
            nc.vector.tensor_reduce(out=lsum, in_=hdiff, op=ALU.add,
                                    axis=mybir.AxisListType.XY)
            nc.sync.dma_start(out=loss_d[:, :], in_=lsum)

    _split_waits(nc)
    return nc


# ---------------------------------------------------------------- entry point

def kernel(triple_emb, W_sfa, W1, b1, W2, b2, tri2path_size):
    _patch_tile_drain()
    triple_emb = np.asarray(triple_emb, np.float32)
    sizes_flat = np.asarray(tri2path_size, np.int32).reshape(-1).astype(np.int64)
    offsets = np.concatenate([[0], np.cumsum(sizes_flat)[:-1]])

    core_rows, bins_all, halves_all = _pack(sizes_flat)
    NB = max(len(b) for b in bins_all)
    NB = ((NB + 3) // 4) * 4
    NP0 = max(h for h in halves_all) // 2
    # all cores must share one program: normalize each core's half boundary
    # by padding half0 with empty bins up to 2*NP0
    for c in range(NCORES):
        h0 = halves_all[c]
        if h0 < 2 * NP0:
            bins_all[c] = (bins_all[c][:h0] + [[]] * (2 * NP0 - h0)
                           + bins_all[c][h0:])
    NB = max(max(len(b) for b in bins_all), NB)
    NB = ((NB + 3) // 4) * 4

    b1_np = np.asarray(b1, np.float32)
    with_b1 = bool(np.any(b1_np != 0.0))

    triple_bf = triple_emb.astype(bf16)
    wsfa_t = np.ascontiguousarray(
        np.asarray(W_sfa, np.float32).T.reshape(KC, 128, D).transpose(1, 0, 2)
        .reshape(128, KC * D)).astype(fp8e4)
    w1_t = np.ascontiguousarray(
        (np.asarray(W1, np.float32) * 8.0).T.reshape(KC, 128, 4 * D)
        .transpose(1, 0, 2).reshape(128, KC * 4 * D)).astype(fp8e4)
    w2_t = np.ascontiguousarray(
        np.repeat((np.asarray(W2, np.float32) * 16.0).reshape(HC, 128).T
                  [:, :, None], 16, axis=2).reshape(128, HC * 16)).astype(fp8e4)
    b1_r = b1_np.reshape(1, HC * 128).astype(bf16)
    b2_r = np.asarray(b2, np.float32).reshape(1, 1)
    in_maps = []
    for c in range(NCORES):
        xm, xt, slot_st = _build_core_arrays(bins_all[c], triple_bf, offsets, NB)
        m = {
            "x_bins": xm, "xt_bins": xt, "slot_of": slot_st,
            "w_sfa_t": wsfa_t, "w1_t": w1_t, "w2_t": w2_t,
            "b2_r": b2_r,
        }
        if with_b1:
            m["b1_r"] = b1_r
        in_maps.append(m)

    with _compile_lock:
        key = (NB, NP0, with_b1)
        nc = _compile_cache.get(key)
        if nc is None:
            nc = _build_program(NB, NP0, with_b1)
            _compile_cache[key] = nc

    res = run_bass_kernel_spmd(nc, in_maps, core_ids=list(range(NCORES)),
                               trace=bool(int(os.environ.get("KGE_TRACE", "0"))))
    total = np.float64(0.0)
    for r in res.results:
        total += np.float64(r["loss"][0, 0])
    total -= np.float64(NCORES * ROWS_PER_CORE * GAMMA)
    kernel.last_results = res
    return np.asarray(np.float32(total))


# revision 41
# speedup vs baseline: 1.0842x; 1.0224x over previous
"""Trainium2 Bass kernel for nn_ContextKGEModel (self-attentive path pooling + FFN hinge loss).

Data-parallel over the 2048 ragged groups, 8 NeuronCores:
  - Host: assign 16 whole batch rows per core (load-balanced), first-fit-
    decreasing-pack each core's 256 groups into 128-row bins, and ship
    triple_emb in two fp8-e4m3 layouts (row-major bins with an appended
    group-mask block, and a transposed copy in supertiles of 4 bins).
    Weights are replicated and pre-transposed; W1 is host-scaled by 8 and
    W2 by 16 (scalings fold into the sigmoid scale). A +/-1 pair-selection
    matrix (bf16) encodes the hinge pairs.
  - Device (per core): PE is kept continuously busy (warm-up matmuls hold the
    p-state clock at max). xwT = W_sfa^T @ X^T per supertile and the per-bin
    Gram run as fp8 DoubleRow matmuls; the group-masked column max runs as a
    single fused tensor_tensor_reduce per bin on DVE (tanh is monotone so it
    commutes with max); tanh+exp are batched per supertile on ACT; attention
    one-hot*exp rows are built on Pool+DVE; the pooled vectors accumulate
    TRANSPOSED in PSUM (lhsT = x rows) along with a ones-row that yields the
    softmax denominators, so no on-chip transposes or normalization pass are
    needed -- relu is positively homogeneous, so the 1/denom scaling folds
    into the final sigmoid step. PSUM->SBUF copies of xwT spread across
    ACT/DVE/Pool. FFN + hinge loss run on-chip; host sums 8 partial losses.
"""

import os
import threading
from contextlib import ExitStack

import numpy as np
import ml_dtypes

import concourse.bass as bass
import concourse.tile as tile
from concourse import mybir
from concourse.vector_clock import ScopedClock
from concourse.bass_utils import run_bass_kernel_spmd
from concourse.masks import make_identity

bf16 = ml_dtypes.bfloat16
fp8 = ml_dtypes.float8_e5m2
fp8e4 = ml_dtypes.float8_e4m3

B, NEG, L, D = 128, 15, 32, 768
NPAIR_SET = 120                      # 240 hinge pairs split into 2 matmul sets
G = B * (NEG + 1)
GAMMA = 0.1
NCORES = 8
ROWS_PER_CORE = B // NCORES          # 16 batch rows / core
SLOTS = ROWS_PER_CORE * (NEG + 1)    # 256 group slots / core
BIN = 128
KC = D // 128                        # 6 contraction chunks
HC = (4 * D) // 128                  # 24 hidden chunks
NEG_MASK = -240.0

_compile_cache = {}
_compile_lock = threading.Lock()


def _patch_tile_drain():
    """This walrus build rejects >1 sem-wait on an instruction ("Too many sync
    wait commands"); split the TileContext tail-drain waits across SP nops."""
    if getattr(tile.TileContext, "_drain_patch_applied", False):
        return

    def _drain_and_barrier(self, tick_clock, wait_clock):
        probe = self.nc.sync.nop(nofuse=True, hint="drain_wait_split")
        wait_clock.add_sem_waits(probe.ins, ScopedClock({None: tick_clock.global_clock}))
        si = probe.ins.sync_info
        waits = list(si.on_wait) if si is not None and si.on_wait else []
        if len(waits) > 1:
            si.on_wait = waits[:1]
            for w in waits[1:]:
                extra = self.nc.sync.nop(nofuse=True, hint="drain_wait_split")
                esi = extra.ins.sync_info
                if esi is None:
                    extra.ins.sync_info = mybir.SyncInfo(on_wait=[w], on_update=[])
                else:
                    esi.on_wait = [w]
        self.nc.sync.drain()
        self.nc.all_engine_barrier()
        assert self.sems is not None
        popped = self.nc._tile_sem_poison_stack.pop()
        assert popped is self._sem_poison
        self.nc.clear_and_free_semaphores(list(self.sems.allocated().values()))
        self.nc.all_engine_barrier()

    tile.TileContext._drain_and_barrier = _drain_and_barrier
    tile.TileContext._drain_patch_applied = True


_MAX_WAITS = 1


def _split_waits(nc, maxw=_MAX_WAITS):
    """Hoist excess sync-waits onto NoOps inserted just before the
    instruction on the same engine (walrus build caps waits/instruction)."""
    n_split = 0
    for fn in nc.m.functions:
        for bb in fn.blocks:
            out = []
            for inst in bb.instructions:
                si = inst.sync_info
                waits = list(si.on_wait) if si is not None and si.on_wait else []
                if len(waits) > maxw:
                    keep = waits[:maxw]
                    rest = waits[maxw:]
                    for i in range(0, len(rest), maxw):
                        n_split += 1
                        nop = mybir.InstNoOp(
                            name=f"WSPLIT-{n_split}",
                            engine=inst.engine,
                            debug=inst.debug,
                            ins=[], outs=[],
                            sync_info=mybir.SyncInfo(
                                on_wait=rest[i:i + maxw], on_update=[]),
                        )
                        out.append(nop)
                    si.on_wait = keep
                out.append(inst)
            if n_split:
                bb.instructions[:] = out
    return n_split


# ---------------------------------------------------------------- host packing

def _pack(sizes_flat):
    """Balanced batch-row -> core assignment, then per-half (8 rows = 128
    slots) first-fit-decreasing bin packing so each bin-pair's groups live in
    one 128-slot window."""
    sizes = sizes_flat.reshape(B, NEG + 1)
    row_load = sizes.sum(1)
    order = np.argsort(-row_load, kind="stable")
    core_rows = [[] for _ in range(NCORES)]
    core_load = np.zeros(NCORES, np.int64)
    for b in order:
        cands = [c for c in range(NCORES) if len(core_rows[c]) < ROWS_PER_CORE]
        c = min(cands, key=lambda c: core_load[c])
        core_rows[c].append(int(b))
        core_load[c] += row_load[b]
    bins_all = []
    halves_all = []
    for c in range(NCORES):
        # split the 16 rows into two halves with balanced total load
        rows_sorted = sorted(core_rows[c], key=lambda b: -row_load[b])
        half_rows = [[], []]
        hl = [0, 0]
        for b in rows_sorted:
            h = 0 if (hl[0] <= hl[1] and len(half_rows[0]) < 8) or                      len(half_rows[1]) >= 8 else 1
            half_rows[h].append(b)
            hl[h] += row_load[b]
        ordered = half_rows[0] + half_rows[1]
        core_rows[c] = ordered
        bins_c = []
        half_sizes = []
        for h in range(2):
            groups = []
            for lh, b in enumerate(half_rows[h]):
                lb = h * 8 + lh
                for k in range(NEG + 1):
                    g = b * (NEG + 1) + k
                    groups.append((g, lb * (NEG + 1) + k, int(sizes_flat[g])))
            groups.sort(key=lambda t: -t[2])
            bins = []
            for g, slot, n in groups:
                for bn in bins:
                    if bn[0] + n <= BIN:
                        bn[1].append((g, slot, n, bn[0]))
                        bn[0] += n
                        break
                else:
                    bins.append([n, [(g, slot, n, 0)]])
            hb = [bn[1] for bn in bins]
            if len(hb) % 2:
                hb.append([])
            half_sizes.append(len(hb))
            bins_c.extend(hb)
        bins_all.append(bins_c)
        halves_all.append(half_sizes[0])
    return core_rows, bins_all, halves_all


def _build_core_arrays(bins_c, triple_bf, offsets, NB):
    """Per-core packed device inputs (supertile-major layouts)."""
    NS = NB // 4
    X = np.zeros((NB, BIN, D), fp8e4)
    gid = np.full((NB, BIN), -1, np.int32)
    slot_of = np.full((NB, BIN), -1, np.int32)
    for bi, bn in enumerate(bins_c):
        for g, slot, n, off in bn:
            X[bi, off:off + n, :] = triple_bf[offsets[g]:offsets[g] + n].astype(fp8e4)
            gid[bi, off:off + n] = g
            slot_of[bi, off:off + n] = slot
    same = (gid[:, :, None] == gid[:, None, :]) & (gid[:, :, None] >= 0)
    m_add = np.where(same, np.float32(0.0), np.float32(NEG_MASK)).astype(fp8e4)
    # supertile-major packings; x + mask merged into one DMA per supertile
    x_st = X.reshape(NS, 4, BIN, D).transpose(0, 2, 1, 3).reshape(NS, BIN, 4 * D)
    madd_st = m_add.reshape(NS, 4, BIN, BIN).transpose(0, 2, 1, 3) \
                   .reshape(NS, BIN, 4 * BIN)
    xm = np.ascontiguousarray(np.concatenate([x_st, madd_st], axis=2))
    xt = np.ascontiguousarray(
        X.reshape(NS, 4, BIN, KC, 128)             # [s, b4, r, c, d]
         .transpose(0, 4, 3, 1, 2)                 # [s, d, c, b4, r]
         .reshape(NS, 128, KC, 4 * BIN))
    slot_st = np.ascontiguousarray(
        slot_of.astype(np.float32).reshape(NS, 4, BIN).transpose(2, 0, 1))  # [BIN,NS,4]
    return xm, xt, slot_st


# ---------------------------------------------------------------- device program

N_WARM = 3          # PE warm-up matmuls (p-state hold through DMA startup)
WARM_F = 384        # free size of each warm-up matmul


def _build_program(NB, NP0, with_b1):
    NS = NB // 4
    NP = NB // 2
    nc = bass.Bass()
    dt = mybir.dt
    AF = mybir.ActivationFunctionType
    ALU = mybir.AluOpType

    XMW = 4 * D + 4 * BIN  # x rows + mask columns, fp8 bytes per partition
    x_d = nc.dram_tensor("x_bins", [NS, BIN, XMW], dt.float8e4, kind="ExternalInput")
    xt_d = nc.dram_tensor("xt_bins", [NS, 128, KC, 4 * BIN], dt.float8e4, kind="ExternalInput")
    slot_d = nc.dram_tensor("slot_of", [BIN, NS, 4], dt.float32, kind="ExternalInput")
    wsfa_d = nc.dram_tensor("w_sfa_t", [128, KC * D], dt.float8e4, kind="ExternalInput")
    w1t_d = nc.dram_tensor("w1_t", [128, KC * 4 * D], dt.float8e4, kind="ExternalInput")
    w2t_d = nc.dram_tensor("w2_t", [128, HC * 16], dt.float8e4, kind="ExternalInput")
    if with_b1:
        b1_d = nc.dram_tensor("b1_r", [1, HC * 128], dt.bfloat16, kind="ExternalInput")
    b2_d = nc.dram_tensor("b2_r", [1, 1], dt.float32, kind="ExternalInput")
    loss_d = nc.dram_tensor("loss", [1, 1], dt.float32, kind="ExternalOutput")
    DEBUG = bool(int(os.environ.get("KGE_DEBUG", "0")))
    if DEBUG:
        dbg_scores_d = nc.dram_tensor("dbg_scores", [1, SLOTS], dt.float32,
                                      kind="ExternalOutput")
        dbg_denom_d = nc.dram_tensor("dbg_denom", [1, SLOTS], dt.float32,
                                     kind="ExternalOutput")
        dbg_pt_d = nc.dram_tensor("dbg_pt", [128, KC, SLOTS], dt.float8e4,
                                  kind="ExternalOutput")
        dbg_att_d = nc.dram_tensor("dbg_att", [128, 2, SLOTS], dt.float8e4,
                                   kind="ExternalOutput")
        dbg_hr_d = nc.dram_tensor("dbg_hr", [128, HC, SLOTS], dt.float8e4,
                                  kind="ExternalOutput")
        dbg_spre_d = nc.dram_tensor("dbg_spre", [1, SLOTS], dt.float32,
                                    kind="ExternalOutput")
        dbg_pss_d = nc.dram_tensor("dbg_pss", [16, SLOTS], dt.float32,
                                   kind="ExternalOutput")

    with tile.TileContext(nc) as tc, ExitStack() as ctx:
        consts = ctx.enter_context(tc.tile_pool(name="consts", bufs=1))
        xres = ctx.enter_context(tc.tile_pool(name="xres", bufs=1))
        attres = ctx.enter_context(tc.tile_pool(name="attres", bufs=1))
        xt_pool = ctx.enter_context(tc.tile_pool(name="xt", bufs=4))
        xwt_pool = ctx.enter_context(tc.tile_pool(name="xwt", bufs=4))
        small = ctx.enter_context(tc.tile_pool(name="small", bufs=12))
        gm_pool = ctx.enter_context(tc.tile_pool(name="gm", bufs=6))
        ffn_pool = ctx.enter_context(tc.tile_pool(name="ffn", bufs=1))

        # resident constants / scratch
        wsfa = consts.tile([128, KC, D], dt.float8e4)      # [d_in_chunk, kc, e]
        slot_all = consts.tile([128, NS, 4], dt.float32)
        iota_i = consts.tile([128, SLOTS], dt.int32)
        nc.gpsimd.iota(iota_i, pattern=[[1, SLOTS]], base=0, channel_multiplier=0)
        iota_f = consts.tile([128, SLOTS], dt.float32)
        nc.vector.tensor_copy(iota_f, iota_i)
        ones2 = consts.tile([128, 2, 16], dt.float8e4)
        nc.vector.memset(ones2, 1.0)

        x_tiles = [xres.tile([128, XMW], dt.float8e4, tag=f"x{s}", name=f"x{s}")
                   for s in range(NS)]
        att_pairs = [attres.tile([128, 2, SLOTS], dt.float8e4, tag=f"a{p}", name=f"a{p}")
                     for p in range(NP)]
        w1t = consts.tile([128, KC, 4 * D], dt.float8e4)
        w2t = consts.tile([128, HC, 16], dt.float8e4)
        if with_b1:
            b1s = consts.tile([1, HC * 128], dt.bfloat16)
        b2s = consts.tile([1, 1], dt.float32)

        # ---- phase A: xwT per supertile; per-bin attention weights one
        # supertile behind; transposed-pooled accumulation two supertiles
        # behind (keeps PE off the ACT/DVE softmax critical path)
        with tc.tile_pool(name="ps_pool", bufs=1, space="PSUM") as ps_pooled, \
             ExitStack() as actx:
            ps_xw = actx.enter_context(tc.tile_pool(name="ps_xw", bufs=3, space="PSUM"))
            ps_gm = actx.enter_context(tc.tile_pool(name="ps_gm", bufs=1, space="PSUM"))
            xt_tiles = {}
            xwt_tiles = {}

            def emit_load(s):
                xt_t = xt_pool.tile([128, KC, 4 * BIN], dt.float8e4, tag="xt",
                                    name=f"xt{s}")
                nc.sync.dma_start(out=xt_t, in_=xt_d[s])
                nc.sync.dma_start(out=x_tiles[s], in_=x_d[s])
                xt_tiles[s] = xt_t

            # startup order: wsfa first (first xw matmul needs it), then the
            # first supertile's tiles, then everything else round-robin
            nc.sync.dma_start(out=wsfa, in_=wsfa_d[:, :].rearrange("p (k e) -> p k e", k=KC))
            emit_load(0)
            nc.sync.dma_start(out=slot_all, in_=slot_d[:, :, :])
            emit_load(1)

            def emit_weight_loads(part):
                w1v = w1t_d[:, :].rearrange("p (k h) -> p k h", k=KC)
                if part == 0:
                    nc.sync.dma_start(out=w1t[:, 0:2, :], in_=w1v[:, 0:2, :])
                    nc.sync.dma_start(out=w2t, in_=w2t_d[:, :].rearrange("p (h r) -> p h r", r=16))
                elif part == 1:
                    nc.sync.dma_start(out=w1t[:, 2:4, :], in_=w1v[:, 2:4, :])
                    if with_b1:
                        nc.sync.dma_start(out=b1s, in_=b1_d[:, :])
                    nc.sync.dma_start(out=b2s, in_=b2_d[:, :])
                else:
                    nc.sync.dma_start(out=w1t[:, 4:6, :], in_=w1v[:, 4:6, :])

            # xwT copy engines per e-chunk: 4x ACT, 2x DVE (Pool can't read PSUM)
            def _copy_xwt(dst, src, e):
                if e in (1, 4):
                    nc.vector.tensor_copy(dst, src)
                else:
                    nc.scalar.copy(dst, src)

            def emit_xw_mm(s):
                xt_t = xt_tiles[s]
                xwt_t = xwt_pool.tile([128, KC, 4 * BIN], dt.float8e4, tag="xwt",
                                      name=f"xwt{s}")
                pss = []
                for e in range(KC):
                    ps = ps_xw.tile([128, 4 * BIN], dt.float32, tag="psxw",
                                    name=f"psxw{s}_{e}")
                    for k in range(0, KC, 2):
                        nc.tensor.matmul(
                            ps, wsfa[:, k:k + 2, e * 128:(e + 1) * 128],
                            xt_t[:, k:k + 2, :],
                            start=(k == 0), stop=(k == KC - 2),
                            perf_mode=mybir.MatmulPerfMode.DoubleRow)
                    pss.append(ps)
                xwt_tiles[s] = xwt_t
                return pss

            def emit_copy(s, pss, e):
                xwt_t = xwt_tiles[s]
                dve = (1, 4) if s % 2 == 0 else (1, 3, 4)
                if e in dve:
                    nc.vector.tensor_copy(xwt_t[:, e, :], pss[e])
                else:
                    nc.scalar.copy(xwt_t[:, e, :], pss[e])

            def emit_gram(s):
                xt_t, xwt_t = xt_tiles[s], xwt_tiles[s]
                ps_g4 = ps_gm.tile([128, 4, BIN], dt.float32, tag="psgm",
                                   name=f"psgm{s}")
                for b in range(4):
                    sl = slice(b * BIN, (b + 1) * BIN)
                    for e in range(0, KC, 2):
                        nc.tensor.matmul(ps_g4[:, b, :], xwt_t[:, e:e + 2, sl],
                                         xt_t[:, e:e + 2, sl],
                                         start=(e == 0), stop=(e == KC - 2),
                                         perf_mode=mybir.MatmulPerfMode.DoubleRow)
                gram_ps[s] = ps_g4

            def emit_mask(s):
                madd_t = x_tiles[s][:, 4 * D:].rearrange("p (j i) -> p j i", i=BIN)
                ps_g4 = gram_ps[s]
                colmax4 = small.tile([128, 4], dt.bfloat16, tag="colmax",
                                     name=f"colmax{s}")
                scratch = gm_pool.tile([128, 4, BIN], dt.bfloat16, tag="gmm",
                                       name=f"gmm{s}")
                nc.vector.tensor_add(scratch, ps_g4, madd_t)
                nc.vector.tensor_reduce(out=colmax4, in_=scratch,
                                        op=ALU.max, axis=mybir.AxisListType.X)
                colmax_t[s] = colmax4

            def emit_acts(s):
                th4 = small.tile([128, 4], dt.float32, tag="th4", name=f"th{s}")
                nc.scalar.activation(th4, colmax_t[s], AF.Tanh)
                ex4 = small.tile([128, 4], dt.float32, tag="ex4", name=f"ex{s}")
                nc.scalar.activation(ex4, th4, AF.Exp)
                ex_t[s] = ex4

            def emit_att(s):
                slot_t = slot_all[:, s, :]
                ex4 = ex_t[s]
                last = (s == NS - 1)
                for b in range(4):
                    pi = s * 2 + b // 2
                    j = b % 2
                    eng = nc.vector if (last and b % 2 == 1) else nc.gpsimd
                    eng.tensor_scalar(
                        out=att_pairs[pi][:, j, :], in0=iota_f,
                        scalar1=slot_t[:, b:b + 1],
                        scalar2=ex4[:, b:b + 1],
                        op0=ALU.is_equal, op1=ALU.mult)

            # transposed pooled accumulation: ps_pT_k[k//2][:, k%2, w] holds
            # sum over rows of x[row, k-chunk] * att[row, slot-window]; the
            # ones-matmul rows in ps_pTd yield the softmax denominators
            ps_pT01 = ps_pooled.tile([128, 2, SLOTS], dt.float32, tag="pspT01",
                                     name="pspT01")
            ps_pT23 = ps_pooled.tile([128, 2, SLOTS], dt.float32, tag="pspT23",
                                     name="pspT23")
            ps_pT45 = ps_pooled.tile([128, 2, SLOTS], dt.float32, tag="pspT45",
                                     name="pspT45")
            ps_pTd = ps_pooled.tile([16, SLOTS], dt.float32, tag="pspTd",
                                    name="pspTd")
            ps_pT_k = [ps_pT01, ps_pT23, ps_pT45]

            def pT(k):
                return ps_pT_k[k // 2][:, k % 2, :]

            def emit_pooled(s):
                xv = x_tiles[s][:, :4 * D].rearrange("p (b w) -> p b w", w=D)
                for bp in range(2):
                    pi = s * 2 + bp
                    att_t = att_pairs[pi]
                    w = 0 if pi < NP0 else 1
                    sl = slice(w * 128, (w + 1) * 128)
                    stop = pi in (NP0 - 1, NP - 1)
                    for k in range(KC):
                        # one start per PSUM bank (first-ever touch); every
                        # other region's first write lands on pending-zero
                        nc.tensor.matmul(
                            pT(k)[:, sl],
                            xv[:, 2 * bp:2 * bp + 2, k * 128:(k + 1) * 128],
                            att_t[:, :, sl],
                            start=(pi == 0 and k % 2 == 0),
                            stop=stop,
                            perf_mode=mybir.MatmulPerfMode.DoubleRow,
                            skip_group_check=True)
                    nc.tensor.matmul(
                        ps_pTd[:, sl], ones2, att_t[:, :, sl],
                        start=(pi == 0), stop=stop,
                        perf_mode=mybir.MatmulPerfMode.DoubleRow,
                        skip_group_check=True)

            gram_ps = {}
            colmax_t = {}
            ex_t = {}
            for s in range(NS):
                pss = emit_xw_mm(s)
                if s >= 1:
                    emit_gram(s - 1)          # PE: after xw so copies of s-1
                                              # are long done
                emit_copy(s, pss, 1)          # DVE (c1 first: bank for e4)
                emit_copy(s, pss, 0)          # ACT
                emit_copy(s, pss, 2)          # ACT
                emit_copy(s, pss, 4)          # DVE (bank for next e1)
                emit_copy(s, pss, 3)          # ACT
                emit_copy(s, pss, 5)          # ACT
                if s >= 1:
                    emit_mask(s - 1)          # DVE add+reduce after c1/c4
                if s >= 1:
                    emit_acts(s - 1)          # ACT tanh+exp after c5
                if s >= 1:
                    emit_att(s - 1)           # Pool
                if s + 2 < NS:
                    emit_load(s + 2)
                if s in (2, 4, 6):
                    emit_weight_loads((s - 2) // 2)
                if s >= 4:
                    emit_pooled(s - 4)
            emit_gram(NS - 1)
            emit_mask(NS - 1)
            emit_acts(NS - 1)
            emit_att(NS - 1)
            for sp in range(NS - 4, NS):
                emit_pooled(sp)
            actx.close()  # release ps_xw / ps_gm banks for phase B

            # ---- phase B1: pooled^T (scaled by 1/8 to keep fp8 range) to
            # SBUF; the denominators to SBUF fp32
            pooledT = ffn_pool.tile([128, KC, SLOTS], dt.float8e4, tag="pooledT")
            for k in range(KC):
                src_ap = pT(k)
                if k % 2 == 0:
                    nc.scalar.activation(pooledT[:, k, :], src_ap,
                                         AF.Copy, scale=0.125)
                else:
                    nc.vector.tensor_scalar(
                        out=pooledT[:, k, :], in0=src_ap,
                        scalar1=0.125, scalar2=None, op0=ALU.mult)
            denom = ffn_pool.tile([1, SLOTS], dt.float32, tag="denom")
            nc.vector.tensor_copy(denom, ps_pTd[0:1, :])
            if DEBUG:
                nc.sync.dma_start(out=dbg_denom_d[:, :], in_=denom)
                nc.sync.dma_start(out=dbg_pt_d[:, :, :], in_=pooledT)
                nc.sync.dma_start(out=dbg_att_d[:, :, :], in_=att_pairs[0])
            rdenom = ffn_pool.tile([1, SLOTS], dt.float32, tag="rdenom")
            nc.vector.reciprocal(rdenom, denom)

        # ---- phase B2: FFN + hinge loss
        with (
            tc.tile_pool(name="ps_h", bufs=4, space="PSUM") as ps_h,
            tc.tile_pool(name="ps_sc", bufs=1, space="PSUM") as ps_sc,
        ):
            # h = relu(W1 @ pooledT [+ denom*b1]); W1 host-scaled 8, pooled
            # scaled 1/8 -> psum holds true pre-activation
            hrelu = ffn_pool.tile([128, HC, SLOTS], dt.float8e4, tag="hrelu")
            ps_s = ps_sc.tile([16, SLOTS], dt.float32, tag="ps_s", name="ps_s")
            for hc in range(HC):
                ps_hh = ps_h.tile([128, SLOTS], dt.float32, tag="psh",
                                  name=f"psh{hc}")
                for k in range(0, KC, 2):
                    nc.tensor.matmul(ps_hh,
                                     w1t[:, k:k + 2, hc * 128:(hc + 1) * 128],
                                     pooledT[:, k:k + 2, :],
                                     start=(k == 0),
                                     stop=(k == KC - 2 and not with_b1),
                                     perf_mode=mybir.MatmulPerfMode.DoubleRow)
                if with_b1:
                    # bias for unnormalized pooled: + denom[slot]*b1[h]
                    nc.tensor.matmul(ps_hh, b1s[:, hc * 128:(hc + 1) * 128],
                                     denom, start=False, stop=True)
                if hc % 2 == 0:
                    nc.scalar.activation(hrelu[:, hc, :], ps_hh, AF.Relu)
                else:
                    nc.vector.tensor_scalar(
                        out=hrelu[:, hc, :], in0=ps_hh, scalar1=0.0,
                        scalar2=None, op0=ALU.max)
                # W2 contraction (DoubleRow over hidden-chunk pairs),
                # interleaved so PE never stalls on the relu chain
                if hc % 2 == 1 and hc >= 5:
                    h2 = hc - 5
                    nc.tensor.matmul(
                        ps_s, w2t[:, h2:h2 + 2, :],
                        hrelu[:, h2:h2 + 2, :],
                        start=(h2 == 0), stop=False,
                        perf_mode=mybir.MatmulPerfMode.DoubleRow)
            for h2 in (HC - 4, HC - 2):
                nc.tensor.matmul(
                    ps_s, w2t[:, h2:h2 + 2, :],
                    hrelu[:, h2:h2 + 2, :],
                    start=False, stop=(h2 == HC - 2),
                    perf_mode=mybir.MatmulPerfMode.DoubleRow)

            # scores = sigmoid(ps_s / (16*denom) + b2); W1*8/8 and W2*16
            # leave psum = 16 * denom * true_score_pre
            spre = ffn_pool.tile([1, SLOTS], dt.float32, tag="spre")
            nc.vector.tensor_tensor(out=spre, in0=ps_s[0:1, :], in1=rdenom,
                                    op=ALU.mult)
            if DEBUG:
                nc.sync.dma_start(out=dbg_hr_d[:, :, :], in_=hrelu)
                nc.sync.dma_start(out=dbg_spre_d[:, :], in_=spre)
                pss_sb = ffn_pool.tile([16, SLOTS], dt.float32, tag="pss_sb")
                nc.vector.tensor_copy(pss_sb, ps_s)
                nc.sync.dma_start(out=dbg_pss_d[:, :], in_=pss_sb)
            scores = ffn_pool.tile([1, SLOTS], dt.float32, tag="scores")
            nc.scalar.activation(scores, spre, AF.Sigmoid, bias=b2s,
                                 scale=0.0625)
            if DEBUG:
                scf = ffn_pool.tile([1, SLOTS], dt.float32, tag="scf")
                nc.vector.tensor_copy(scf, scores)
                nc.sync.dma_start(out=dbg_scores_d[:, :], in_=scf)

            # hinge: per-slot relu(s - p_own_row + gamma) via a stride-0
            # broadcast AP (positive slots contribute exactly gamma each;
            # the host subtracts that constant from the summed loss)
            sc_ap = scores[0:1, :]
            p_bcast = bass.AP(tensor=sc_ap.tensor, offset=sc_ap.offset,
                              ap=[[sc_ap.ap[0][0], 1], [16, ROWS_PER_CORE],
                                  [0, NEG + 1]])
            hdiff = ffn_pool.tile([1, ROWS_PER_CORE, NEG + 1], dt.float32,
                                  tag="hdiff")
            nc.vector.tensor_tensor(
                out=hdiff, in0=sc_ap.rearrange("p (a b) -> p a b", b=NEG + 1),
                in1=p_bcast, op=ALU.subtract)
            nc.vector.tensor_scalar(out=hdiff, in0=hdiff, scalar1=GAMMA,
                                    scalar2=0.0, op0=ALU.add, op1=ALU.max)
            lsum = ffn_pool.tile([1, 1], dt.float32, tag="lsum")
            nc.vector.tensor_reduce(out=lsum, in_=hdiff, op=ALU.add,
                                    axis=mybir.AxisListType.XY)
            nc.sync.dma_start(out=loss_d[:, :], in_=lsum)

    _split_waits(nc)
    return nc


# ---------------------------------------------------------------- entry point

def kernel(triple_emb, W_sfa, W1, b1, W2, b2, tri2path_size):
    _patch_tile_drain()
    triple_emb = np.asarray(triple_emb, np.float32)
    sizes_flat = np.asarray(tri2path_size, np.int32).reshape(-1).astype(np.int64)
    offsets = np.concatenate([[0], np.cumsum(sizes_flat)[:-1]])

    core_rows, bins_all, halves_all = _pack(sizes_flat)
    NB = max(len(b) for b in bins_all)
    NB = ((NB + 3) // 4) * 4
    NP0 = max(h for h in halves_all) // 2
    # all cores must share one program: normalize each core's half boundary
    # by padding half0 with empty bins up to 2*NP0
    for c in range(NCORES):
        h0 = halves_all[c]
        if h0 < 2 * NP0:
            bins_all[c] = (bins_all[c][:h0] + [[]] * (2 * NP0 - h0)
                           + bins_all[c][h0:])
    NB = max(max(len(b) for b in bins_all), NB)
    NB = ((NB + 3) // 4) * 4

    b1_np = np.asarray(b1, np.float32)
    with_b1 = bool(np.any(b1_np != 0.0))

    triple_bf = triple_emb.astype(bf16)
    wsfa_t = np.ascontiguousarray(
        np.asarray(W_sfa, np.float32).T.reshape(KC, 128, D).transpose(1, 0, 2)
        .reshape(128, KC * D)).astype(fp8e4)
    w1_t = np.ascontiguousarray(
        (np.asarray(W1, np.float32) * 8.0).T.reshape(KC, 128, 4 * D)
        .transpose(1, 0, 2).reshape(128, KC * 4 * D)).astype(fp8e4)
    w2_t = np.ascontiguousarray(
        np.repeat((np.asarray(W2, np.float32) * 16.0).reshape(HC, 128).T
                  [:, :, None], 16, axis=2).reshape(128, HC * 16)).astype(fp8e4)
    b1_r = b1_np.reshape(1, HC * 128).astype(bf16)
    b2_r = np.asarray(b2, np.float32).reshape(1, 1)
    in_maps = []
    for c in range(NCORES):
        xm, xt, slot_st = _build_core_arrays(bins_all[c], triple_bf, offsets, NB)
        m = {
            "x_bins": xm, "xt_bins": xt, "slot_of": slot_st,
            "w_sfa_t": wsfa_t, "w1_t": w1_t, "w2_t": w2_t,
            "b2_r": b2_r,
        }
        if with_b1:
            m["b1_r"] = b1_r
        in_maps.append(m)

    with _compile_lock:
        key = (NB, NP0, with_b1)
        nc = _compile_cache.get(key)
        if nc is None:
            nc = _build_program(NB, NP0, with_b1)
            _compile_cache[key] = nc

    res = run_bass_kernel_spmd(nc, in_maps, core_ids=list(range(NCORES)),
                               trace=bool(int(os.environ.get("KGE_TRACE", "0"))))
    total = np.float64(0.0)
    for r in res.results:
        total += np.float64(r["loss"][0, 0])
    total -= np.float64(NCORES * ROWS_PER_CORE * GAMMA)
    kernel.last_results = res
    return np.asarray(np.float32(total))


# revision 42
# speedup vs baseline: 1.1405x; 1.0519x over previous
"""Trainium2 Bass kernel for nn_ContextKGEModel (self-attentive path pooling + FFN hinge loss).

Data-parallel over the 2048 ragged groups, 8 NeuronCores:
  - Host: assign 16 whole batch rows per core (load-balanced), first-fit-
    decreasing-pack each core's 256 groups into 128-row bins, and ship
    triple_emb in two fp8-e4m3 layouts (row-major bins with an appended
    group-mask block, and a transposed copy in supertiles of 4 bins).
    Weights are replicated and pre-transposed; W1 is host-scaled by 8 and
    W2 by 16 (scalings fold into the sigmoid scale). A +/-1 pair-selection
    matrix (bf16) encodes the hinge pairs.
  - Device (per core): PE is kept continuously busy (warm-up matmuls hold the
    p-state clock at max). xwT = W_sfa^T @ X^T per supertile and the per-bin
    Gram run as fp8 DoubleRow matmuls; the group-masked column max runs as a
    single fused tensor_tensor_reduce per bin on DVE (tanh is monotone so it
    commutes with max); tanh+exp are batched per supertile on ACT; attention
    one-hot*exp rows are built on Pool+DVE; the pooled vectors accumulate
    TRANSPOSED in PSUM (lhsT = x rows) along with a ones-row that yields the
    softmax denominators, so no on-chip transposes or normalization pass are
    needed -- relu is positively homogeneous, so the 1/denom scaling folds
    into the final sigmoid step. PSUM->SBUF copies of xwT spread across
    ACT/DVE/Pool. FFN + hinge loss run on-chip; host sums 8 partial losses.
"""

import os
import threading
from contextlib import ExitStack

import numpy as np
import ml_dtypes

import concourse.bass as bass
import concourse.tile as tile
from concourse import mybir
from concourse.vector_clock import ScopedClock
from concourse.bass_utils import run_bass_kernel_spmd
from concourse.masks import make_identity

bf16 = ml_dtypes.bfloat16
fp8 = ml_dtypes.float8_e5m2
fp8e4 = ml_dtypes.float8_e4m3

B, NEG, L, D = 128, 15, 32, 768
NPAIR_SET = 120                      # 240 hinge pairs split into 2 matmul sets
G = B * (NEG + 1)
GAMMA = 0.1
NCORES = 8
ROWS_PER_CORE = B // NCORES          # 16 batch rows / core
SLOTS = ROWS_PER_CORE * (NEG + 1)    # 256 group slots / core
BIN = 128
KC = D // 128                        # 6 contraction chunks
HC = (4 * D) // 128                  # 24 hidden chunks
NEG_MASK = -240.0

_compile_cache = {}
_compile_lock = threading.Lock()


def _patch_tile_drain():
    """This walrus build rejects >1 sem-wait on an instruction ("Too many sync
    wait commands"); split the TileContext tail-drain waits across SP nops."""
    if getattr(tile.TileContext, "_drain_patch_applied", False):
        return

    def _drain_and_barrier(self, tick_clock, wait_clock):
        probe = self.nc.sync.nop(nofuse=True, hint="drain_wait_split")
        wait_clock.add_sem_waits(probe.ins, ScopedClock({None: tick_clock.global_clock}))
        si = probe.ins.sync_info
        waits = list(si.on_wait) if si is not None and si.on_wait else []
        if len(waits) > 1:
            si.on_wait = waits[:1]
            for w in waits[1:]:
                extra = self.nc.sync.nop(nofuse=True, hint="drain_wait_split")
                esi = extra.ins.sync_info
                if esi is None:
                    extra.ins.sync_info = mybir.SyncInfo(on_wait=[w], on_update=[])
                else:
                    esi.on_wait = [w]
        self.nc.sync.drain()
        self.nc.all_engine_barrier()
        assert self.sems is not None
        popped = self.nc._tile_sem_poison_stack.pop()
        assert popped is self._sem_poison
        self.nc.clear_and_free_semaphores(list(self.sems.allocated().values()))
        self.nc.all_engine_barrier()

    tile.TileContext._drain_and_barrier = _drain_and_barrier
    tile.TileContext._drain_patch_applied = True


_MAX_WAITS = 1


def _split_waits(nc, maxw=_MAX_WAITS):
    """Hoist excess sync-waits onto NoOps inserted just before the
    instruction on the same engine (walrus build caps waits/instruction)."""
    n_split = 0
    for fn in nc.m.functions:
        for bb in fn.blocks:
            out = []
            for inst in bb.instructions:
                si = inst.sync_info
                waits = list(si.on_wait) if si is not None and si.on_wait else []
                if len(waits) > maxw:
                    keep = waits[:maxw]
                    rest = waits[maxw:]
                    for i in range(0, len(rest), maxw):
                        n_split += 1
                        nop = mybir.InstNoOp(
                            name=f"WSPLIT-{n_split}",
                            engine=inst.engine,
                            debug=inst.debug,
                            ins=[], outs=[],
                            sync_info=mybir.SyncInfo(
                                on_wait=rest[i:i + maxw], on_update=[]),
                        )
                        out.append(nop)
                    si.on_wait = keep
                out.append(inst)
            if n_split:
                bb.instructions[:] = out
    return n_split


# ---------------------------------------------------------------- host packing

def _pack(sizes_flat):
    """Balanced batch-row -> core assignment, then per-half (8 rows = 128
    slots) first-fit-decreasing bin packing so each bin-pair's groups live in
    one 128-slot window."""
    sizes = sizes_flat.reshape(B, NEG + 1)
    row_load = sizes.sum(1)
    order = np.argsort(-row_load, kind="stable")
    core_rows = [[] for _ in range(NCORES)]
    core_load = np.zeros(NCORES, np.int64)
    for b in order:
        cands = [c for c in range(NCORES) if len(core_rows[c]) < ROWS_PER_CORE]
        c = min(cands, key=lambda c: core_load[c])
        core_rows[c].append(int(b))
        core_load[c] += row_load[b]
    bins_all = []
    halves_all = []
    for c in range(NCORES):
        # split the 16 rows into two halves with balanced total load
        rows_sorted = sorted(core_rows[c], key=lambda b: -row_load[b])
        half_rows = [[], []]
        hl = [0, 0]
        for b in rows_sorted:
            h = 0 if (hl[0] <= hl[1] and len(half_rows[0]) < 8) or                      len(half_rows[1]) >= 8 else 1
            half_rows[h].append(b)
            hl[h] += row_load[b]
        ordered = half_rows[0] + half_rows[1]
        core_rows[c] = ordered
        bins_c = []
        half_sizes = []
        for h in range(2):
            groups = []
            for lh, b in enumerate(half_rows[h]):
                lb = h * 8 + lh
                for k in range(NEG + 1):
                    g = b * (NEG + 1) + k
                    groups.append((g, lb * (NEG + 1) + k, int(sizes_flat[g])))
            groups.sort(key=lambda t: -t[2])
            bins = []
            for g, slot, n in groups:
                for bn in bins:
                    if bn[0] + n <= BIN:
                        bn[1].append((g, slot, n, bn[0]))
                        bn[0] += n
                        break
                else:
                    bins.append([n, [(g, slot, n, 0)]])
            hb = [bn[1] for bn in bins]
            if len(hb) % 2:
                hb.append([])
            half_sizes.append(len(hb))
            bins_c.extend(hb)
        bins_all.append(bins_c)
        halves_all.append(half_sizes[0])
    return core_rows, bins_all, halves_all


def _build_core_arrays(bins_c, triple_bf, offsets, NB):
    """Per-core packed device inputs (supertile-major layouts)."""
    NS = NB // 4
    X = np.zeros((NB, BIN, D), fp8e4)
    gid = np.full((NB, BIN), -1, np.int32)
    slot_of = np.full((NB, BIN), -1, np.int32)
    for bi, bn in enumerate(bins_c):
        for g, slot, n, off in bn:
            X[bi, off:off + n, :] = triple_bf[offsets[g]:offsets[g] + n].astype(fp8e4)
            gid[bi, off:off + n] = g
            slot_of[bi, off:off + n] = slot
    same = (gid[:, :, None] == gid[:, None, :]) & (gid[:, :, None] >= 0)
    m_add = np.where(same, np.float32(0.0), np.float32(NEG_MASK)).astype(fp8e4)
    # supertile-major packings; x + mask merged into one DMA per supertile
    x_st = X.reshape(NS, 4, BIN, D).transpose(0, 2, 1, 3).reshape(NS, BIN, 4 * D)
    madd_st = m_add.reshape(NS, 4, BIN, BIN).transpose(0, 2, 1, 3) \
                   .reshape(NS, BIN, 4 * BIN)
    xm = np.ascontiguousarray(np.concatenate([x_st, madd_st], axis=2))
    xt = np.ascontiguousarray(
        X.reshape(NS, 4, BIN, KC, 128)             # [s, b4, r, c, d]
         .transpose(0, 4, 3, 1, 2)                 # [s, d, c, b4, r]
         .reshape(NS, 128, KC, 4 * BIN))
    slot_st = np.ascontiguousarray(
        slot_of.astype(np.float32).reshape(NS, 4, BIN).transpose(2, 0, 1))  # [BIN,NS,4]
    return xm, xt, slot_st


# ---------------------------------------------------------------- device program

N_WARM = 3          # PE warm-up matmuls (p-state hold through DMA startup)
WARM_F = 384        # free size of each warm-up matmul


def _build_program(NB, NP0, with_b1):
    NS = NB // 4
    NP = NB // 2
    nc = bass.Bass()
    dt = mybir.dt
    AF = mybir.ActivationFunctionType
    ALU = mybir.AluOpType

    XMW = 4 * D + 4 * BIN  # x rows + mask columns, fp8 bytes per partition
    x_d = nc.dram_tensor("x_bins", [NS, BIN, XMW], dt.float8e4, kind="ExternalInput")
    xt_d = nc.dram_tensor("xt_bins", [NS, 128, KC, 4 * BIN], dt.float8e4, kind="ExternalInput")
    slot_d = nc.dram_tensor("slot_of", [BIN, NS, 4], dt.float32, kind="ExternalInput")
    wsfa_d = nc.dram_tensor("w_sfa_t", [128, KC * D], dt.float8e4, kind="ExternalInput")
    w1t_d = nc.dram_tensor("w1_t", [128, KC * 4 * D], dt.float8e4, kind="ExternalInput")
    w2t_d = nc.dram_tensor("w2_t", [128, HC * 16], dt.float8e4, kind="ExternalInput")
    if with_b1:
        b1_d = nc.dram_tensor("b1_r", [1, HC * 128], dt.bfloat16, kind="ExternalInput")
    b2_d = nc.dram_tensor("b2_r", [1, 1], dt.float32, kind="ExternalInput")
    loss_d = nc.dram_tensor("loss", [1, 1], dt.float32, kind="ExternalOutput")
    DEBUG = bool(int(os.environ.get("KGE_DEBUG", "0")))
    if DEBUG:
        dbg_scores_d = nc.dram_tensor("dbg_scores", [1, SLOTS], dt.float32,
                                      kind="ExternalOutput")
        dbg_denom_d = nc.dram_tensor("dbg_denom", [1, SLOTS], dt.float32,
                                     kind="ExternalOutput")
        dbg_pt_d = nc.dram_tensor("dbg_pt", [128, KC, SLOTS], dt.float8e4,
                                  kind="ExternalOutput")
        dbg_att_d = nc.dram_tensor("dbg_att", [128, 2, SLOTS], dt.float8e4,
                                   kind="ExternalOutput")
        dbg_hr_d = nc.dram_tensor("dbg_hr", [128, HC, SLOTS], dt.float8e4,
                                  kind="ExternalOutput")
        dbg_spre_d = nc.dram_tensor("dbg_spre", [1, SLOTS], dt.float32,
                                    kind="ExternalOutput")
        dbg_pss_d = nc.dram_tensor("dbg_pss", [16, SLOTS], dt.float32,
                                   kind="ExternalOutput")

    with tile.TileContext(nc) as tc, ExitStack() as ctx:
        consts = ctx.enter_context(tc.tile_pool(name="consts", bufs=1))
        xres = ctx.enter_context(tc.tile_pool(name="xres", bufs=1))
        attres = ctx.enter_context(tc.tile_pool(name="attres", bufs=1))
        xt_pool = ctx.enter_context(tc.tile_pool(name="xt", bufs=6))
        xwt_pool = ctx.enter_context(tc.tile_pool(name="xwt", bufs=6))
        small = ctx.enter_context(tc.tile_pool(name="small", bufs=12))
        gm_pool = ctx.enter_context(tc.tile_pool(name="gm", bufs=6))
        ffn_pool = ctx.enter_context(tc.tile_pool(name="ffn", bufs=1))

        # resident constants / scratch
        wsfa = consts.tile([128, KC, D], dt.float8e4)      # [d_in_chunk, kc, e]
        slot_all = consts.tile([128, NS, 4], dt.float32)
        iota_i = consts.tile([128, SLOTS], dt.int32)
        nc.gpsimd.iota(iota_i, pattern=[[1, SLOTS]], base=0, channel_multiplier=0)
        iota_f = consts.tile([128, SLOTS], dt.float32)
        nc.vector.tensor_copy(iota_f, iota_i)
        ones2 = consts.tile([128, 2, 16], dt.float8e4)
        nc.vector.memset(ones2, 1.0)

        x_tiles = [xres.tile([128, XMW], dt.float8e4, tag=f"x{s}", name=f"x{s}")
                   for s in range(NS)]
        att_pairs = [attres.tile([128, 2, SLOTS], dt.float8e4, tag=f"a{p}", name=f"a{p}")
                     for p in range(NP)]
        w1t = consts.tile([128, KC, 4 * D], dt.float8e4)
        w2t = consts.tile([128, HC, 16], dt.float8e4)
        if with_b1:
            b1s = consts.tile([1, HC * 128], dt.bfloat16)
        b2s = consts.tile([1, 1], dt.float32)

        # ---- phase A: xwT per supertile; per-bin attention weights one
        # supertile behind; transposed-pooled accumulation two supertiles
        # behind (keeps PE off the ACT/DVE softmax critical path)
        with tc.tile_pool(name="ps_pool", bufs=1, space="PSUM") as ps_pooled, \
             ExitStack() as actx:
            ps_xw = actx.enter_context(tc.tile_pool(name="ps_xw", bufs=3, space="PSUM"))
            ps_gm = actx.enter_context(tc.tile_pool(name="ps_gm", bufs=1, space="PSUM"))
            xt_tiles = {}
            xwt_tiles = {}

            def emit_load(s):
                xt_t = xt_pool.tile([128, KC, 4 * BIN], dt.float8e4, tag="xt",
                                    name=f"xt{s}")
                nc.sync.dma_start(out=xt_t, in_=xt_d[s])
                nc.sync.dma_start(out=x_tiles[s], in_=x_d[s])
                xt_tiles[s] = xt_t

            # startup order: wsfa first (first xw matmul needs it), then the
            # first supertile's tiles, then everything else round-robin
            nc.sync.dma_start(out=wsfa, in_=wsfa_d[:, :].rearrange("p (k e) -> p k e", k=KC))
            emit_load(0)
            nc.sync.dma_start(out=slot_all, in_=slot_d[:, :, :])
            emit_load(1)

            def emit_weight_loads(part):
                w1v = w1t_d[:, :].rearrange("p (k h) -> p k h", k=KC)
                if part == 0:
                    nc.sync.dma_start(out=w1t[:, 0:2, :], in_=w1v[:, 0:2, :])
                    nc.sync.dma_start(out=w2t, in_=w2t_d[:, :].rearrange("p (h r) -> p h r", r=16))
                    if with_b1:
                        nc.sync.dma_start(out=b1s, in_=b1_d[:, :])
                    nc.sync.dma_start(out=b2s, in_=b2_d[:, :])
                elif part == 1:
                    nc.sync.dma_start(out=w1t[:, 2:4, :], in_=w1v[:, 2:4, :])
                else:
                    nc.sync.dma_start(out=w1t[:, 4:6, :], in_=w1v[:, 4:6, :])

            # xwT copy engines per e-chunk: 4x ACT, 2x DVE (Pool can't read PSUM)
            def _copy_xwt(dst, src, e):
                if e in (1, 4):
                    nc.vector.tensor_copy(dst, src)
                else:
                    nc.scalar.copy(dst, src)

            def emit_xw_mm(s):
                xt_t = xt_tiles[s]
                xwt_t = xwt_pool.tile([128, KC, 4 * BIN], dt.float8e4, tag="xwt",
                                      name=f"xwt{s}")
                pss = []
                for e in range(KC):
                    ps = ps_xw.tile([128, 4 * BIN], dt.float32, tag="psxw",
                                    name=f"psxw{s}_{e}")
                    for k in range(0, KC, 2):
                        nc.tensor.matmul(
                            ps, wsfa[:, k:k + 2, e * 128:(e + 1) * 128],
                            xt_t[:, k:k + 2, :],
                            start=(k == 0), stop=(k == KC - 2),
                            perf_mode=mybir.MatmulPerfMode.DoubleRow)
                    pss.append(ps)
                xwt_tiles[s] = xwt_t
                return pss

            def emit_copy(s, pss, e):
                xwt_t = xwt_tiles[s]
                dve = (1, 4) if s % 2 == 0 else (1, 3, 4)
                if e in dve:
                    nc.vector.tensor_copy(xwt_t[:, e, :], pss[e])
                else:
                    nc.scalar.copy(xwt_t[:, e, :], pss[e])

            def emit_gram(s):
                xt_t, xwt_t = xt_tiles[s], xwt_tiles[s]
                ps_g4 = ps_gm.tile([128, 4, BIN], dt.float32, tag="psgm",
                                   name=f"psgm{s}")
                for b in range(4):
                    sl = slice(b * BIN, (b + 1) * BIN)
                    for e in range(0, KC, 2):
                        nc.tensor.matmul(ps_g4[:, b, :], xwt_t[:, e:e + 2, sl],
                                         xt_t[:, e:e + 2, sl],
                                         start=(e == 0), stop=(e == KC - 2),
                                         perf_mode=mybir.MatmulPerfMode.DoubleRow)
                gram_ps[s] = ps_g4

            def emit_mask(s):
                madd_t = x_tiles[s][:, 4 * D:].rearrange("p (j i) -> p j i", i=BIN)
                ps_g4 = gram_ps[s]
                colmax4 = small.tile([128, 4], dt.bfloat16, tag="colmax",
                                     name=f"colmax{s}")
                scratch = gm_pool.tile([128, 4, BIN], dt.bfloat16, tag="gmm",
                                       name=f"gmm{s}")
                nc.vector.tensor_add(scratch, ps_g4, madd_t)
                nc.vector.tensor_reduce(out=colmax4, in_=scratch,
                                        op=ALU.max, axis=mybir.AxisListType.X)
                colmax_t[s] = colmax4

            def emit_acts(s):
                th4 = small.tile([128, 4], dt.float32, tag="th4", name=f"th{s}")
                nc.scalar.activation(th4, colmax_t[s], AF.Tanh)
                ex4 = small.tile([128, 4], dt.float32, tag="ex4", name=f"ex{s}")
                nc.scalar.activation(ex4, th4, AF.Exp)
                ex_t[s] = ex4

            def emit_att(s):
                slot_t = slot_all[:, s, :]
                ex4 = ex_t[s]
                last = (s == NS - 1)
                for b in range(4):
                    pi = s * 2 + b // 2
                    j = b % 2
                    eng = nc.vector if (last and b % 2 == 1) else nc.gpsimd
                    eng.tensor_scalar(
                        out=att_pairs[pi][:, j, :], in0=iota_f,
                        scalar1=slot_t[:, b:b + 1],
                        scalar2=ex4[:, b:b + 1],
                        op0=ALU.is_equal, op1=ALU.mult)

            # transposed pooled accumulation: ps_pT_k[k//2][:, k%2, w] holds
            # sum over rows of x[row, k-chunk] * att[row, slot-window]; the
            # ones-matmul rows in ps_pTd yield the softmax denominators
            ps_pT01 = ps_pooled.tile([128, 2, SLOTS], dt.float32, tag="pspT01",
                                     name="pspT01")
            ps_pT23 = ps_pooled.tile([128, 2, SLOTS], dt.float32, tag="pspT23",
                                     name="pspT23")
            ps_pT45 = ps_pooled.tile([128, 2, SLOTS], dt.float32, tag="pspT45",
                                     name="pspT45")
            ps_pTd = ps_pooled.tile([16, SLOTS], dt.float32, tag="pspTd",
                                    name="pspTd")
            ps_pT_k = [ps_pT01, ps_pT23, ps_pT45]

            def pT(k):
                return ps_pT_k[k // 2][:, k % 2, :]

            def emit_pooled(s):
                xv = x_tiles[s][:, :4 * D].rearrange("p (b w) -> p b w", w=D)
                for bp in range(2):
                    pi = s * 2 + bp
                    att_t = att_pairs[pi]
                    w = 0 if pi < NP0 else 1
                    sl = slice(w * 128, (w + 1) * 128)
                    stop = pi in (NP0 - 1, NP - 1)
                    for k in range(KC):
                        # one start per PSUM bank (first-ever touch); every
                        # other region's first write lands on pending-zero
                        nc.tensor.matmul(
                            pT(k)[:, sl],
                            xv[:, 2 * bp:2 * bp + 2, k * 128:(k + 1) * 128],
                            att_t[:, :, sl],
                            start=(pi == 0 and k % 2 == 0),
                            stop=stop,
                            perf_mode=mybir.MatmulPerfMode.DoubleRow,
                            skip_group_check=True)
                    nc.tensor.matmul(
                        ps_pTd[:, sl], ones2, att_t[:, :, sl],
                        start=(pi == 0), stop=stop,
                        perf_mode=mybir.MatmulPerfMode.DoubleRow,
                        skip_group_check=True)

            gram_ps = {}
            colmax_t = {}
            ex_t = {}
            for s in range(NS):
                pss = emit_xw_mm(s)
                if s >= 1:
                    emit_gram(s - 1)          # PE: after xw so copies of s-1
                                              # are long done
                emit_copy(s, pss, 1)          # DVE (c1 first: bank for e4)
                emit_copy(s, pss, 0)          # ACT
                emit_copy(s, pss, 2)          # ACT
                emit_copy(s, pss, 4)          # DVE (bank for next e1)
                emit_copy(s, pss, 3)          # ACT
                emit_copy(s, pss, 5)          # ACT
                if s >= 1:
                    emit_mask(s - 1)          # DVE add+reduce after c1/c4
                if s >= 1:
                    emit_acts(s - 1)          # ACT tanh+exp after c5
                if s >= 1:
                    emit_att(s - 1)           # Pool
                if s + 2 < NS:
                    emit_load(s + 2)
                if s in (5, 7, 8):
                    emit_weight_loads((5, 7, 8).index(s))
                if s >= 4:
                    emit_pooled(s - 4)
            emit_gram(NS - 1)
            # per-pair attention chain for the last supertile so the phase-B
            # entry isn't gated on the full-supertile latency
            sL = NS - 1
            madd_L = x_tiles[sL][:, 4 * D:].rearrange("p (j i) -> p j i", i=BIN)
            slot_L = slot_all[:, sL, :]
            for sp in range(NS - 4, NS - 1):
                emit_pooled(sp)
            for bp in range(2):
                cm2 = small.tile([128, 2], dt.bfloat16, tag="colmax",
                                 name=f"cmL{bp}")
                sc2 = gm_pool.tile([128, 2, BIN], dt.bfloat16, tag="gmm",
                                   name=f"gmL{bp}")
                nc.vector.tensor_add(sc2, gram_ps[sL][:, 2 * bp:2 * bp + 2, :],
                                     madd_L[:, 2 * bp:2 * bp + 2, :])
                nc.vector.tensor_reduce(out=cm2, in_=sc2, op=ALU.max,
                                        axis=mybir.AxisListType.X)
                th2 = small.tile([128, 2], dt.float32, tag="th4", name=f"thL{bp}")
                nc.scalar.activation(th2, cm2, AF.Tanh)
                ex2 = small.tile([128, 2], dt.float32, tag="ex4", name=f"exL{bp}")
                nc.scalar.activation(ex2, th2, AF.Exp)
                pi = sL * 2 + bp
                for j in range(2):
                    eng = nc.gpsimd if j == 0 else nc.vector
                    eng.tensor_scalar(
                        out=att_pairs[pi][:, j, :], in0=iota_f,
                        scalar1=slot_L[:, 2 * bp + j:2 * bp + j + 1],
                        scalar2=ex2[:, j:j + 1],
                        op0=ALU.is_equal, op1=ALU.mult)
            emit_pooled(NS - 1)
            actx.close()  # release ps_xw / ps_gm banks for phase B

            # ---- phase B1: pooled^T (scaled by 1/8 to keep fp8 range) to
            # SBUF; the denominators to SBUF fp32
            pooledT = ffn_pool.tile([128, KC, SLOTS], dt.float8e4, tag="pooledT")
            for k in range(KC):
                src_ap = pT(k)
                if k % 2 == 0:
                    nc.scalar.activation(pooledT[:, k, :], src_ap,
                                         AF.Copy, scale=0.125)
                else:
                    nc.vector.tensor_scalar(
                        out=pooledT[:, k, :], in0=src_ap,
                        scalar1=0.125, scalar2=None, op0=ALU.mult)
            denom = ffn_pool.tile([1, SLOTS], dt.float32, tag="denom")
            nc.vector.tensor_copy(denom, ps_pTd[0:1, :])
            if DEBUG:
                nc.sync.dma_start(out=dbg_denom_d[:, :], in_=denom)
                nc.sync.dma_start(out=dbg_pt_d[:, :, :], in_=pooledT)
                nc.sync.dma_start(out=dbg_att_d[:, :, :], in_=att_pairs[0])
            rdenom = ffn_pool.tile([1, SLOTS], dt.float32, tag="rdenom")
            nc.vector.reciprocal(rdenom, denom)

        # ---- phase B2: FFN + hinge loss
        with (
            tc.tile_pool(name="ps_h", bufs=4, space="PSUM") as ps_h,
            tc.tile_pool(name="ps_sc", bufs=1, space="PSUM") as ps_sc,
        ):
            # h = relu(W1 @ pooledT [+ denom*b1]); W1 host-scaled 8, pooled
            # scaled 1/8 -> psum holds true pre-activation
            hrelu = ffn_pool.tile([128, HC, SLOTS], dt.float8e4, tag="hrelu")
            ps_s = ps_sc.tile([16, SLOTS], dt.float32, tag="ps_s", name="ps_s")
            for hc in range(HC):
                ps_hh = ps_h.tile([128, SLOTS], dt.float32, tag="psh",
                                  name=f"psh{hc}")
                for k in range(0, KC, 2):
                    nc.tensor.matmul(ps_hh,
                                     w1t[:, k:k + 2, hc * 128:(hc + 1) * 128],
                                     pooledT[:, k:k + 2, :],
                                     start=(k == 0),
                                     stop=(k == KC - 2 and not with_b1),
                                     perf_mode=mybir.MatmulPerfMode.DoubleRow)
                if with_b1:
                    # bias for unnormalized pooled: + denom[slot]*b1[h]
                    nc.tensor.matmul(ps_hh, b1s[:, hc * 128:(hc + 1) * 128],
                                     denom, start=False, stop=True)
                if hc % 2 == 0:
                    nc.scalar.activation(hrelu[:, hc, :], ps_hh, AF.Relu)
                else:
                    nc.vector.tensor_scalar(
                        out=hrelu[:, hc, :], in0=ps_hh, scalar1=0.0,
                        scalar2=None, op0=ALU.max)
                # W2 contraction (DoubleRow over hidden-chunk pairs),
                # interleaved so PE never stalls on the relu chain
                if hc % 2 == 1 and hc >= 5:
                    h2 = hc - 5
                    nc.tensor.matmul(
                        ps_s, w2t[:, h2:h2 + 2, :],
                        hrelu[:, h2:h2 + 2, :],
                        start=(h2 == 0), stop=False,
                        perf_mode=mybir.MatmulPerfMode.DoubleRow)
            for h2 in (HC - 4, HC - 2):
                nc.tensor.matmul(
                    ps_s, w2t[:, h2:h2 + 2, :],
                    hrelu[:, h2:h2 + 2, :],
                    start=False, stop=(h2 == HC - 2),
                    perf_mode=mybir.MatmulPerfMode.DoubleRow)

            # scores = sigmoid(ps_s / (16*denom) + b2); W1*8/8 and W2*16
            # leave psum = 16 * denom * true_score_pre
            spre = ffn_pool.tile([1, SLOTS], dt.float32, tag="spre")
            nc.vector.tensor_tensor(out=spre, in0=ps_s[0:1, :], in1=rdenom,
                                    op=ALU.mult)
            if DEBUG:
                nc.sync.dma_start(out=dbg_hr_d[:, :, :], in_=hrelu)
                nc.sync.dma_start(out=dbg_spre_d[:, :], in_=spre)
                pss_sb = ffn_pool.tile([16, SLOTS], dt.float32, tag="pss_sb")
                nc.vector.tensor_copy(pss_sb, ps_s)
                nc.sync.dma_start(out=dbg_pss_d[:, :], in_=pss_sb)
            scores = ffn_pool.tile([1, SLOTS], dt.float32, tag="scores")
            nc.scalar.activation(scores, spre, AF.Sigmoid, bias=b2s,
                                 scale=0.0625)
            if DEBUG:
                scf = ffn_pool.tile([1, SLOTS], dt.float32, tag="scf")
                nc.vector.tensor_copy(scf, scores)
                nc.sync.dma_start(out=dbg_scores_d[:, :], in_=scf)

            # hinge: per-slot relu(s - p_own_row + gamma) via a stride-0
            # broadcast AP (positive slots contribute exactly gamma each;
            # the host subtracts that constant from the summed loss)
            sc_ap = scores[0:1, :]
            p_bcast = bass.AP(tensor=sc_ap.tensor, offset=sc_ap.offset,
                              ap=[[sc_ap.ap[0][0], 1], [16, ROWS_PER_CORE],
                                  [0, NEG + 1]])
            hdiff = ffn_pool.tile([1, ROWS_PER_CORE, NEG + 1], dt.float32,
                                  tag="hdiff")
            nc.vector.tensor_tensor(
                out=hdiff, in0=sc_ap.rearrange("p (a b) -> p a b", b=NEG + 1),
                in1=p_bcast, op=ALU.subtract)
            nc.vector.tensor_scalar(out=hdiff, in0=hdiff, scalar1=GAMMA,
                                    scalar2=0.0, op0=ALU.add, op1=ALU.max)
            lsum = ffn_pool.tile([1, 1], dt.float32, tag="lsum")
            nc.vector.tensor_reduce(out=lsum, in_=hdiff, op=ALU.add,
                                    axis=mybir.AxisListType.XY)
            nc.sync.dma_start(out=loss_d[:, :], in_=lsum)

    _split_waits(nc)
    return nc


# ---------------------------------------------------------------- entry point

def kernel(triple_emb, W_sfa, W1, b1, W2, b2, tri2path_size):
    _patch_tile_drain()
    triple_emb = np.asarray(triple_emb, np.float32)
    sizes_flat = np.asarray(tri2path_size, np.int32).reshape(-1).astype(np.int64)
    offsets = np.concatenate([[0], np.cumsum(sizes_flat)[:-1]])

    core_rows, bins_all, halves_all = _pack(sizes_flat)
    NB = max(len(b) for b in bins_all)
    NB = ((NB + 3) // 4) * 4
    NP0 = max(h for h in halves_all) // 2
    # all cores must share one program: normalize each core's half boundary
    # by padding half0 with empty bins up to 2*NP0
    for c in range(NCORES):
        h0 = halves_all[c]
        if h0 < 2 * NP0:
            bins_all[c] = (bins_all[c][:h0] + [[]] * (2 * NP0 - h0)
                           + bins_all[c][h0:])
    NB = max(max(len(b) for b in bins_all), NB)
    NB = ((NB + 3) // 4) * 4

    b1_np = np.asarray(b1, np.float32)
    with_b1 = bool(np.any(b1_np != 0.0))

    triple_bf = triple_emb.astype(bf16)
    wsfa_t = np.ascontiguousarray(
        np.asarray(W_sfa, np.float32).T.reshape(KC, 128, D).transpose(1, 0, 2)
        .reshape(128, KC * D)).astype(fp8e4)
    w1_t = np.ascontiguousarray(
        (np.asarray(W1, np.float32) * 8.0).T.reshape(KC, 128, 4 * D)
        .transpose(1, 0, 2).reshape(128, KC * 4 * D)).astype(fp8e4)
    w2_t = np.ascontiguousarray(
        np.repeat((np.asarray(W2, np.float32) * 16.0).reshape(HC, 128).T
                  [:, :, None], 16, axis=2).reshape(128, HC * 16)).astype(fp8e4)
    b1_r = b1_np.reshape(1, HC * 128).astype(bf16)
    b2_r = np.asarray(b2, np.float32).reshape(1, 1)
    in_maps = []
    for c in range(NCORES):
        xm, xt, slot_st = _build_core_arrays(bins_all[c], triple_bf, offsets, NB)
        m = {
            "x_bins": xm, "xt_bins": xt, "slot_of": slot_st,
            "w_sfa_t": wsfa_t, "w1_t": w1_t, "w2_t": w2_t,
            "b2_r": b2_r,
        }
        if with_b1:
            m["b1_r"] = b1_r
        in_maps.append(m)

    with _compile_lock:
        key = (NB, NP0, with_b1)
        nc = _compile_cache.get(key)
        if nc is None:
            nc = _build_program(NB, NP0, with_b1)
            _compile_cache[key] = nc

    res = run_bass_kernel_spmd(nc, in_maps, core_ids=list(range(NCORES)),
                               trace=bool(int(os.environ.get("KGE_TRACE", "0"))))
    total = np.float64(0.0)
    for r in res.results:
        total += np.float64(r["loss"][0, 0])
    total -= np.float64(NCORES * ROWS_PER_CORE * GAMMA)
    kernel.last_results = res
    return np.asarray(np.float32(total))


# revision 43
# speedup vs baseline: 1.1554x; 1.0131x over previous
"""Trainium2 Bass kernel for nn_ContextKGEModel (self-attentive path pooling + FFN hinge loss).

Data-parallel over the 2048 ragged groups, 8 NeuronCores:
  - Host: assign 16 whole batch rows per core (load-balanced), first-fit-
    decreasing-pack each core's 256 groups into 128-row bins, and ship
    triple_emb in two fp8-e4m3 layouts (row-major bins with an appended
    group-mask block, and a transposed copy in supertiles of 4 bins).
    Weights are replicated and pre-transposed; W1 is host-scaled by 8 and
    W2 by 16 (scalings fold into the sigmoid scale). A +/-1 pair-selection
    matrix (bf16) encodes the hinge pairs.
  - Device (per core): PE is kept continuously busy (warm-up matmuls hold the
    p-state clock at max). xwT = W_sfa^T @ X^T per supertile and the per-bin
    Gram run as fp8 DoubleRow matmuls; the group-masked column max runs as a
    single fused tensor_tensor_reduce per bin on DVE (tanh is monotone so it
    commutes with max); tanh+exp are batched per supertile on ACT; attention
    one-hot*exp rows are built on Pool+DVE; the pooled vectors accumulate
    TRANSPOSED in PSUM (lhsT = x rows) along with a ones-row that yields the
    softmax denominators, so no on-chip transposes or normalization pass are
    needed -- relu is positively homogeneous, so the 1/denom scaling folds
    into the final sigmoid step. PSUM->SBUF copies of xwT spread across
    ACT/DVE/Pool. FFN + hinge loss run on-chip; host sums 8 partial losses.
"""

import os
import threading
from contextlib import ExitStack

import numpy as np
import ml_dtypes

import concourse.bass as bass
import concourse.tile as tile
from concourse import mybir
from concourse.vector_clock import ScopedClock
from concourse.bass_utils import run_bass_kernel_spmd
from concourse.masks import make_identity

bf16 = ml_dtypes.bfloat16
fp8 = ml_dtypes.float8_e5m2
fp8e4 = ml_dtypes.float8_e4m3

B, NEG, L, D = 128, 15, 32, 768
NPAIR_SET = 120                      # 240 hinge pairs split into 2 matmul sets
G = B * (NEG + 1)
GAMMA = 0.1
NCORES = 8
ROWS_PER_CORE = B // NCORES          # 16 batch rows / core
SLOTS = ROWS_PER_CORE * (NEG + 1)    # 256 group slots / core
BIN = 128
KC = D // 128                        # 6 contraction chunks
HC = (4 * D) // 128                  # 24 hidden chunks
NEG_MASK = -240.0

_compile_cache = {}
_compile_lock = threading.Lock()


def _patch_tile_drain():
    """This walrus build rejects >1 sem-wait on an instruction ("Too many sync
    wait commands"); split the TileContext tail-drain waits across SP nops."""
    if getattr(tile.TileContext, "_drain_patch_applied", False):
        return

    def _drain_and_barrier(self, tick_clock, wait_clock):
        probe = self.nc.sync.nop(nofuse=True, hint="drain_wait_split")
        wait_clock.add_sem_waits(probe.ins, ScopedClock({None: tick_clock.global_clock}))
        si = probe.ins.sync_info
        waits = list(si.on_wait) if si is not None and si.on_wait else []
        if len(waits) > 1:
            si.on_wait = waits[:1]
            for w in waits[1:]:
                extra = self.nc.sync.nop(nofuse=True, hint="drain_wait_split")
                esi = extra.ins.sync_info
                if esi is None:
                    extra.ins.sync_info = mybir.SyncInfo(on_wait=[w], on_update=[])
                else:
                    esi.on_wait = [w]
        self.nc.sync.drain()
        self.nc.all_engine_barrier()
        assert self.sems is not None
        popped = self.nc._tile_sem_poison_stack.pop()
        assert popped is self._sem_poison
        self.nc.clear_and_free_semaphores(list(self.sems.allocated().values()))
        self.nc.all_engine_barrier()

    tile.TileContext._drain_and_barrier = _drain_and_barrier
    tile.TileContext._drain_patch_applied = True


_MAX_WAITS = 1


def _split_waits(nc, maxw=_MAX_WAITS):
    """Hoist excess sync-waits onto NoOps inserted just before the
    instruction on the same engine (walrus build caps waits/instruction)."""
    n_split = 0
    for fn in nc.m.functions:
        for bb in fn.blocks:
            out = []
            for inst in bb.instructions:
                si = inst.sync_info
                waits = list(si.on_wait) if si is not None and si.on_wait else []
                if len(waits) > maxw:
                    keep = waits[:maxw]
                    rest = waits[maxw:]
                    for i in range(0, len(rest), maxw):
                        n_split += 1
                        nop = mybir.InstNoOp(
                            name=f"WSPLIT-{n_split}",
                            engine=inst.engine,
                            debug=inst.debug,
                            ins=[], outs=[],
                            sync_info=mybir.SyncInfo(
                                on_wait=rest[i:i + maxw], on_update=[]),
                        )
                        out.append(nop)
                    si.on_wait = keep
                out.append(inst)
            if n_split:
                bb.instructions[:] = out
    return n_split


# ---------------------------------------------------------------- host packing

def _pack(sizes_flat):
    """Balanced batch-row -> core assignment, then per-half (8 rows = 128
    slots) first-fit-decreasing bin packing so each bin-pair's groups live in
    one 128-slot window."""
    sizes = sizes_flat.reshape(B, NEG + 1)
    row_load = sizes.sum(1)
    order = np.argsort(-row_load, kind="stable")
    core_rows = [[] for _ in range(NCORES)]
    core_load = np.zeros(NCORES, np.int64)
    for b in order:
        cands = [c for c in range(NCORES) if len(core_rows[c]) < ROWS_PER_CORE]
        c = min(cands, key=lambda c: core_load[c])
        core_rows[c].append(int(b))
        core_load[c] += row_load[b]
    bins_all = []
    halves_all = []
    for c in range(NCORES):
        # split the 16 rows into two halves with balanced total load
        rows_sorted = sorted(core_rows[c], key=lambda b: -row_load[b])
        half_rows = [[], []]
        hl = [0, 0]
        for b in rows_sorted:
            h = 0 if (hl[0] <= hl[1] and len(half_rows[0]) < 8) or                      len(half_rows[1]) >= 8 else 1
            half_rows[h].append(b)
            hl[h] += row_load[b]
        ordered = half_rows[0] + half_rows[1]
        core_rows[c] = ordered
        bins_c = []
        half_sizes = []
        for h in range(2):
            groups = []
            for lh, b in enumerate(half_rows[h]):
                lb = h * 8 + lh
                for k in range(NEG + 1):
                    g = b * (NEG + 1) + k
                    groups.append((g, lb * (NEG + 1) + k, int(sizes_flat[g])))
            groups.sort(key=lambda t: -t[2])
            bins = []
            for g, slot, n in groups:
                for bn in bins:
                    if bn[0] + n <= BIN:
                        bn[1].append((g, slot, n, bn[0]))
                        bn[0] += n
                        break
                else:
                    bins.append([n, [(g, slot, n, 0)]])
            hb = [bn[1] for bn in bins]
            if len(hb) % 2:
                hb.append([])
            half_sizes.append(len(hb))
            bins_c.extend(hb)
        bins_all.append(bins_c)
        halves_all.append(half_sizes[0])
    return core_rows, bins_all, halves_all


def _build_core_arrays(bins_c, triple_bf, offsets, NB):
    """Per-core packed device inputs (supertile-major layouts)."""
    NS = NB // 4
    X = np.zeros((NB, BIN, D), fp8e4)
    gid = np.full((NB, BIN), -1, np.int32)
    slot_of = np.full((NB, BIN), -1, np.int32)
    for bi, bn in enumerate(bins_c):
        for g, slot, n, off in bn:
            X[bi, off:off + n, :] = triple_bf[offsets[g]:offsets[g] + n].astype(fp8e4)
            gid[bi, off:off + n] = g
            slot_of[bi, off:off + n] = slot
    same = (gid[:, :, None] == gid[:, None, :]) & (gid[:, :, None] >= 0)
    m_add = np.where(same, np.float32(0.0), np.float32(NEG_MASK)).astype(fp8e4)
    # supertile-major packings; x + mask merged into one DMA per supertile
    x_st = X.reshape(NS, 4, BIN, D).transpose(0, 2, 1, 3).reshape(NS, BIN, 4 * D)
    madd_st = m_add.reshape(NS, 4, BIN, BIN).transpose(0, 2, 1, 3) \
                   .reshape(NS, BIN, 4 * BIN)
    xm = np.ascontiguousarray(np.concatenate([x_st, madd_st], axis=2))
    xt = np.ascontiguousarray(
        X.reshape(NS, 4, BIN, KC, 128)             # [s, b4, r, c, d]
         .transpose(0, 4, 3, 1, 2)                 # [s, d, c, b4, r]
         .reshape(NS, 128, KC, 4 * BIN))
    slot_st = np.ascontiguousarray(
        slot_of.astype(np.float32).reshape(NS, 4, BIN).transpose(2, 0, 1))  # [BIN,NS,4]
    return xm, xt, slot_st


# ---------------------------------------------------------------- device program

N_WARM = 3          # PE warm-up matmuls (p-state hold through DMA startup)
WARM_F = 384        # free size of each warm-up matmul


def _build_program(NB, NP0, with_b1):
    NS = NB // 4
    NP = NB // 2
    nc = bass.Bass()
    dt = mybir.dt
    AF = mybir.ActivationFunctionType
    ALU = mybir.AluOpType

    XMW = 4 * D + 4 * BIN  # x rows + mask columns, fp8 bytes per partition
    x_d = nc.dram_tensor("x_bins", [NS, BIN, XMW], dt.float8e4, kind="ExternalInput")
    xt_d = nc.dram_tensor("xt_bins", [NS, 128, KC, 4 * BIN], dt.float8e4, kind="ExternalInput")
    slot_d = nc.dram_tensor("slot_of", [BIN, NS, 4], dt.float32, kind="ExternalInput")
    wsfa_d = nc.dram_tensor("w_sfa_t", [128, KC * D], dt.float8e4, kind="ExternalInput")
    w1t_d = nc.dram_tensor("w1_t", [128, KC * 4 * D], dt.float8e4, kind="ExternalInput")
    w2t_d = nc.dram_tensor("w2_t", [128, HC * 16], dt.float8e4, kind="ExternalInput")
    if with_b1:
        b1_d = nc.dram_tensor("b1_r", [1, HC * 128], dt.bfloat16, kind="ExternalInput")
    b2_d = nc.dram_tensor("b2_r", [1, 1], dt.float32, kind="ExternalInput")
    loss_d = nc.dram_tensor("loss", [1, 1], dt.float32, kind="ExternalOutput")
    DEBUG = bool(int(os.environ.get("KGE_DEBUG", "0")))
    if DEBUG:
        dbg_scores_d = nc.dram_tensor("dbg_scores", [1, SLOTS], dt.float32,
                                      kind="ExternalOutput")
        dbg_denom_d = nc.dram_tensor("dbg_denom", [1, SLOTS], dt.float32,
                                     kind="ExternalOutput")
        dbg_pt_d = nc.dram_tensor("dbg_pt", [128, KC, SLOTS], dt.float8e4,
                                  kind="ExternalOutput")
        dbg_att_d = nc.dram_tensor("dbg_att", [128, 2, SLOTS], dt.float8e4,
                                   kind="ExternalOutput")
        dbg_hr_d = nc.dram_tensor("dbg_hr", [128, HC, SLOTS], dt.float8e4,
                                  kind="ExternalOutput")
        dbg_spre_d = nc.dram_tensor("dbg_spre", [1, SLOTS], dt.float32,
                                    kind="ExternalOutput")
        dbg_pss_d = nc.dram_tensor("dbg_pss", [16, SLOTS], dt.float32,
                                   kind="ExternalOutput")

    with tile.TileContext(nc) as tc, ExitStack() as ctx:
        consts = ctx.enter_context(tc.tile_pool(name="consts", bufs=1))
        xres = ctx.enter_context(tc.tile_pool(name="xres", bufs=1))
        attres = ctx.enter_context(tc.tile_pool(name="attres", bufs=1))
        xt_pool = ctx.enter_context(tc.tile_pool(name="xt", bufs=6))
        xwt_pool = ctx.enter_context(tc.tile_pool(name="xwt", bufs=6))
        small = ctx.enter_context(tc.tile_pool(name="small", bufs=12))
        gm_pool = ctx.enter_context(tc.tile_pool(name="gm", bufs=6))
        ffn_pool = ctx.enter_context(tc.tile_pool(name="ffn", bufs=1))

        # resident constants / scratch
        wsfa = consts.tile([128, KC, D], dt.float8e4)      # [d_in_chunk, kc, e]
        slot_all = consts.tile([128, NS, 4], dt.float32)
        iota_i = consts.tile([128, SLOTS], dt.int32)
        nc.gpsimd.iota(iota_i, pattern=[[1, SLOTS]], base=0, channel_multiplier=0)
        iota_f = consts.tile([128, SLOTS], dt.float32)
        nc.vector.tensor_copy(iota_f, iota_i)
        ones2 = consts.tile([128, 2, 16], dt.float8e4)
        nc.vector.memset(ones2, 1.0)
        identf8 = consts.tile([128, 128], dt.float8e4)
        make_identity(nc, identf8)

        x_tiles = [xres.tile([128, XMW], dt.float8e4, tag=f"x{s}", name=f"x{s}")
                   for s in range(NS)]
        att_pairs = [attres.tile([128, 2, SLOTS], dt.float8e4, tag=f"a{p}", name=f"a{p}")
                     for p in range(NP)]
        w1t = consts.tile([128, KC, 4 * D], dt.float8e4)
        w2t = consts.tile([128, HC, 16], dt.float8e4)
        if with_b1:
            b1s = consts.tile([1, HC * 128], dt.bfloat16)
        b2s = consts.tile([1, 1], dt.float32)

        # ---- phase A: xwT per supertile; per-bin attention weights one
        # supertile behind; transposed-pooled accumulation two supertiles
        # behind (keeps PE off the ACT/DVE softmax critical path)
        with tc.tile_pool(name="ps_pool", bufs=1, space="PSUM") as ps_pooled, \
             ExitStack() as actx:
            ps_xw = actx.enter_context(tc.tile_pool(name="ps_xw", bufs=3, space="PSUM"))
            ps_gm = actx.enter_context(tc.tile_pool(name="ps_gm", bufs=1, space="PSUM"))
            xt_tiles = {}
            xwt_tiles = {}

            def emit_load(s):
                xt_t = xt_pool.tile([128, KC, 4 * BIN], dt.float8e4, tag="xt",
                                    name=f"xt{s}")
                nc.sync.dma_start(out=xt_t, in_=xt_d[s])
                nc.sync.dma_start(out=x_tiles[s], in_=x_d[s])
                xt_tiles[s] = xt_t

            # startup order: wsfa first (first xw matmul needs it), then the
            # first supertile's tiles, then everything else round-robin
            nc.sync.dma_start(out=wsfa, in_=wsfa_d[:, :].rearrange("p (k e) -> p k e", k=KC))
            emit_load(0)
            nc.sync.dma_start(out=slot_all, in_=slot_d[:, :, :])
            emit_load(1)

            def emit_weight_loads(part):
                w1v = w1t_d[:, :].rearrange("p (k h) -> p k h", k=KC)
                if part == 0:
                    nc.sync.dma_start(out=w1t[:, 0:2, :], in_=w1v[:, 0:2, :])
                    nc.sync.dma_start(out=w2t, in_=w2t_d[:, :].rearrange("p (h r) -> p h r", r=16))
                    if with_b1:
                        nc.sync.dma_start(out=b1s, in_=b1_d[:, :])
                    nc.sync.dma_start(out=b2s, in_=b2_d[:, :])
                elif part == 1:
                    nc.sync.dma_start(out=w1t[:, 2:4, :], in_=w1v[:, 2:4, :])
                else:
                    nc.sync.dma_start(out=w1t[:, 4:6, :], in_=w1v[:, 4:6, :])

            # xwT copy engines per e-chunk: 4x ACT, 2x DVE (Pool can't read PSUM)
            def _copy_xwt(dst, src, e):
                if e in (1, 4):
                    nc.vector.tensor_copy(dst, src)
                else:
                    nc.scalar.copy(dst, src)

            def emit_xw_mm(s):
                xt_t = xt_tiles[s]
                xwt_t = xwt_pool.tile([128, KC, 4 * BIN], dt.float8e4, tag="xwt",
                                      name=f"xwt{s}")
                pss = []
                for e in range(KC):
                    ps = ps_xw.tile([128, 4 * BIN], dt.float32, tag="psxw",
                                    name=f"psxw{s}_{e}")
                    for k in range(0, KC, 2):
                        nc.tensor.matmul(
                            ps, wsfa[:, k:k + 2, e * 128:(e + 1) * 128],
                            xt_t[:, k:k + 2, :],
                            start=(k == 0), stop=(k == KC - 2),
                            perf_mode=mybir.MatmulPerfMode.DoubleRow)
                    pss.append(ps)
                xwt_tiles[s] = xwt_t
                return pss

            def emit_copy(s, pss, e):
                xwt_t = xwt_tiles[s]
                if e in (1, 3, 4):
                    nc.vector.tensor_copy(xwt_t[:, e, :], pss[e])
                else:
                    nc.scalar.copy(xwt_t[:, e, :], pss[e])

            def emit_gram(s):
                xt_t, xwt_t = xt_tiles[s], xwt_tiles[s]
                madd_t = x_tiles[s][:, 4 * D:].rearrange("p (j i) -> p j i", i=BIN)
                ps_g4 = ps_gm.tile([128, 4, BIN], dt.float32, tag="psgm",
                                   name=f"psgm{s}")
                for b in range(4):
                    sl = slice(b * BIN, (b + 1) * BIN)
                    # fold the group mask into the accumulation: I @ madd
                    nc.tensor.matmul(ps_g4[:, b, :], identf8, madd_t[:, b, :],
                                     start=True, stop=False,
                                     skip_group_check=True)
                    for e in range(0, KC, 2):
                        nc.tensor.matmul(ps_g4[:, b, :], xwt_t[:, e:e + 2, sl],
                                         xt_t[:, e:e + 2, sl],
                                         start=False, stop=(e == KC - 2),
                                         perf_mode=mybir.MatmulPerfMode.DoubleRow,
                                         skip_group_check=True)
                gram_ps[s] = ps_g4

            def emit_mask(s):
                ps_g4 = gram_ps[s]
                colmax4 = small.tile([128, 4], dt.bfloat16, tag="colmax",
                                     name=f"colmax{s}")
                nc.vector.tensor_reduce(out=colmax4, in_=ps_g4,
                                        op=ALU.max, axis=mybir.AxisListType.X)
                colmax_t[s] = colmax4

            def emit_acts(s):
                th4 = small.tile([128, 4], dt.float32, tag="th4", name=f"th{s}")
                nc.scalar.activation(th4, colmax_t[s], AF.Tanh)
                ex4 = small.tile([128, 4], dt.float32, tag="ex4", name=f"ex{s}")
                nc.scalar.activation(ex4, th4, AF.Exp)
                ex_t[s] = ex4

            def emit_att(s):
                slot_t = slot_all[:, s, :]
                ex4 = ex_t[s]
                last = (s == NS - 1)
                for b in range(4):
                    pi = s * 2 + b // 2
                    j = b % 2
                    eng = nc.vector if (last and b % 2 == 1) else nc.gpsimd
                    eng.tensor_scalar(
                        out=att_pairs[pi][:, j, :], in0=iota_f,
                        scalar1=slot_t[:, b:b + 1],
                        scalar2=ex4[:, b:b + 1],
                        op0=ALU.is_equal, op1=ALU.mult)

            # transposed pooled accumulation: ps_pT_k[k//2][:, k%2, w] holds
            # sum over rows of x[row, k-chunk] * att[row, slot-window]; the
            # ones-matmul rows in ps_pTd yield the softmax denominators
            ps_pT01 = ps_pooled.tile([128, 2, SLOTS], dt.float32, tag="pspT01",
                                     name="pspT01")
            ps_pT23 = ps_pooled.tile([128, 2, SLOTS], dt.float32, tag="pspT23",
                                     name="pspT23")
            ps_pT45 = ps_pooled.tile([128, 2, SLOTS], dt.float32, tag="pspT45",
                                     name="pspT45")
            ps_pTd = ps_pooled.tile([16, SLOTS], dt.float32, tag="pspTd",
                                    name="pspTd")
            ps_pT_k = [ps_pT01, ps_pT23, ps_pT45]

            def pT(k):
                return ps_pT_k[k // 2][:, k % 2, :]

            def emit_pooled(s):
                xv = x_tiles[s][:, :4 * D].rearrange("p (b w) -> p b w", w=D)
                for bp in range(2):
                    pi = s * 2 + bp
                    att_t = att_pairs[pi]
                    w = 0 if pi < NP0 else 1
                    sl = slice(w * 128, (w + 1) * 128)
                    stop = pi in (NP0 - 1, NP - 1)
                    for k in range(KC):
                        # one start per PSUM bank (first-ever touch); every
                        # other region's first write lands on pending-zero
                        nc.tensor.matmul(
                            pT(k)[:, sl],
                            xv[:, 2 * bp:2 * bp + 2, k * 128:(k + 1) * 128],
                            att_t[:, :, sl],
                            start=(pi == 0 and k % 2 == 0),
                            stop=stop,
                            perf_mode=mybir.MatmulPerfMode.DoubleRow,
                            skip_group_check=True)
                    nc.tensor.matmul(
                        ps_pTd[:, sl], ones2, att_t[:, :, sl],
                        start=(pi == 0), stop=stop,
                        perf_mode=mybir.MatmulPerfMode.DoubleRow,
                        skip_group_check=True)

            gram_ps = {}
            colmax_t = {}
            ex_t = {}
            for s in range(NS):
                pss = emit_xw_mm(s)
                if s >= 1:
                    emit_gram(s - 1)          # PE: after xw so copies of s-1
                                              # are long done
                emit_copy(s, pss, 1)          # DVE (c1 first: bank for e4)
                emit_copy(s, pss, 0)          # ACT
                emit_copy(s, pss, 2)          # ACT
                emit_copy(s, pss, 4)          # DVE (bank for next e1)
                emit_copy(s, pss, 3)          # ACT
                emit_copy(s, pss, 5)          # ACT
                if s >= 1:
                    emit_mask(s - 1)          # DVE add+reduce after c1/c4
                if s >= 1:
                    emit_acts(s - 1)          # ACT tanh+exp after c5
                if s >= 1:
                    emit_att(s - 1)           # Pool
                if s + 2 < NS:
                    emit_load(s + 2)
                if s in (5, 7, 8):
                    emit_weight_loads((5, 7, 8).index(s))
                if s >= 4:
                    emit_pooled(s - 4)
            emit_gram(NS - 1)
            # per-pair attention chain for the last supertile so the phase-B
            # entry isn't gated on the full-supertile latency
            sL = NS - 1
            madd_L = x_tiles[sL][:, 4 * D:].rearrange("p (j i) -> p j i", i=BIN)
            slot_L = slot_all[:, sL, :]
            for sp in range(NS - 4, NS - 1):
                emit_pooled(sp)
            for bp in range(2):
                cm2 = small.tile([128, 2], dt.bfloat16, tag="colmax",
                                 name=f"cmL{bp}")
                nc.vector.tensor_reduce(out=cm2,
                                        in_=gram_ps[sL][:, 2 * bp:2 * bp + 2, :],
                                        op=ALU.max, axis=mybir.AxisListType.X)
                th2 = small.tile([128, 2], dt.float32, tag="th4", name=f"thL{bp}")
                nc.scalar.activation(th2, cm2, AF.Tanh)
                ex2 = small.tile([128, 2], dt.float32, tag="ex4", name=f"exL{bp}")
                nc.scalar.activation(ex2, th2, AF.Exp)
                pi = sL * 2 + bp
                for j in range(2):
                    eng = nc.gpsimd if j == 0 else nc.vector
                    eng.tensor_scalar(
                        out=att_pairs[pi][:, j, :], in0=iota_f,
                        scalar1=slot_L[:, 2 * bp + j:2 * bp + j + 1],
                        scalar2=ex2[:, j:j + 1],
                        op0=ALU.is_equal, op1=ALU.mult)
            emit_pooled(NS - 1)
            actx.close()  # release ps_xw / ps_gm banks for phase B

            # ---- phase B1: pooled^T (scaled by 1/8 to keep fp8 range) to
            # SBUF; the denominators to SBUF fp32
            pooledT = ffn_pool.tile([128, KC, SLOTS], dt.float8e4, tag="pooledT")
            for k in range(KC):
                src_ap = pT(k)
                if k % 2 == 0:
                    nc.scalar.activation(pooledT[:, k, :], src_ap,
                                         AF.Copy, scale=0.125)
                else:
                    nc.vector.tensor_scalar(
                        out=pooledT[:, k, :], in0=src_ap,
                        scalar1=0.125, scalar2=None, op0=ALU.mult)
            denom = ffn_pool.tile([1, SLOTS], dt.float32, tag="denom")
            nc.vector.tensor_copy(denom, ps_pTd[0:1, :])
            if DEBUG:
                nc.sync.dma_start(out=dbg_denom_d[:, :], in_=denom)
                nc.sync.dma_start(out=dbg_pt_d[:, :, :], in_=pooledT)
                nc.sync.dma_start(out=dbg_att_d[:, :, :], in_=att_pairs[0])
            rdenom = ffn_pool.tile([1, SLOTS], dt.float32, tag="rdenom")
            nc.vector.reciprocal(rdenom, denom)

        # ---- phase B2: FFN + hinge loss
        with (
            tc.tile_pool(name="ps_h", bufs=4, space="PSUM") as ps_h,
            tc.tile_pool(name="ps_sc", bufs=1, space="PSUM") as ps_sc,
        ):
            # h = relu(W1 @ pooledT [+ denom*b1]); W1 host-scaled 8, pooled
            # scaled 1/8 -> psum holds true pre-activation
            hrelu = ffn_pool.tile([128, HC, SLOTS], dt.float8e4, tag="hrelu")
            ps_s = ps_sc.tile([16, SLOTS], dt.float32, tag="ps_s", name="ps_s")
            for hc in range(HC):
                ps_hh = ps_h.tile([128, SLOTS], dt.float32, tag="psh",
                                  name=f"psh{hc}")
                for k in range(0, KC, 2):
                    nc.tensor.matmul(ps_hh,
                                     w1t[:, k:k + 2, hc * 128:(hc + 1) * 128],
                                     pooledT[:, k:k + 2, :],
                                     start=(k == 0),
                                     stop=(k == KC - 2 and not with_b1),
                                     perf_mode=mybir.MatmulPerfMode.DoubleRow)
                if with_b1:
                    # bias for unnormalized pooled: + denom[slot]*b1[h]
                    nc.tensor.matmul(ps_hh, b1s[:, hc * 128:(hc + 1) * 128],
                                     denom, start=False, stop=True)
                if hc % 2 == 0:
                    nc.scalar.activation(hrelu[:, hc, :], ps_hh, AF.Relu)
                else:
                    nc.vector.tensor_scalar(
                        out=hrelu[:, hc, :], in0=ps_hh, scalar1=0.0,
                        scalar2=None, op0=ALU.max)
                # W2 contraction (DoubleRow over hidden-chunk pairs),
                # interleaved so PE never stalls on the relu chain
                if hc % 2 == 1 and hc >= 5:
                    h2 = hc - 5
                    nc.tensor.matmul(
                        ps_s, w2t[:, h2:h2 + 2, :],
                        hrelu[:, h2:h2 + 2, :],
                        start=(h2 == 0), stop=False,
                        perf_mode=mybir.MatmulPerfMode.DoubleRow)
            for h2 in (HC - 4, HC - 2):
                nc.tensor.matmul(
                    ps_s, w2t[:, h2:h2 + 2, :],
                    hrelu[:, h2:h2 + 2, :],
                    start=False, stop=(h2 == HC - 2),
                    perf_mode=mybir.MatmulPerfMode.DoubleRow)

            # scores = sigmoid(ps_s / (16*denom) + b2); W1*8/8 and W2*16
            # leave psum = 16 * denom * true_score_pre
            spre = ffn_pool.tile([1, SLOTS], dt.float32, tag="spre")
            nc.vector.tensor_tensor(out=spre, in0=ps_s[0:1, :], in1=rdenom,
                                    op=ALU.mult)
            if DEBUG:
                nc.sync.dma_start(out=dbg_hr_d[:, :, :], in_=hrelu)
                nc.sync.dma_start(out=dbg_spre_d[:, :], in_=spre)
                pss_sb = ffn_pool.tile([16, SLOTS], dt.float32, tag="pss_sb")
                nc.vector.tensor_copy(pss_sb, ps_s)
                nc.sync.dma_start(out=dbg_pss_d[:, :], in_=pss_sb)
            scores = ffn_pool.tile([1, SLOTS], dt.float32, tag="scores")
            nc.scalar.activation(scores, spre, AF.Sigmoid, bias=b2s,
                                 scale=0.0625)
            if DEBUG:
                scf = ffn_pool.tile([1, SLOTS], dt.float32, tag="scf")
                nc.vector.tensor_copy(scf, scores)
                nc.sync.dma_start(out=dbg_scores_d[:, :], in_=scf)

            # hinge: per-slot relu(s - p_own_row + gamma) via a stride-0
            # broadcast AP (positive slots contribute exactly gamma each;
            # the host subtracts that constant from the summed loss)
            sc_ap = scores[0:1, :]
            p_bcast = bass.AP(tensor=sc_ap.tensor, offset=sc_ap.offset,
                              ap=[[sc_ap.ap[0][0], 1], [16, ROWS_PER_CORE],
                                  [0, NEG + 1]])
            hdiff = ffn_pool.tile([1, ROWS_PER_CORE, NEG + 1], dt.float32,
                                  tag="hdiff")
            nc.vector.tensor_tensor(
                out=hdiff, in0=sc_ap.rearrange("p (a b) -> p a b", b=NEG + 1),
                in1=p_bcast, op=ALU.subtract)
            nc.vector.tensor_scalar(out=hdiff, in0=hdiff, scalar1=GAMMA,
                                    scalar2=0.0, op0=ALU.add, op1=ALU.max)
            lsum = ffn_pool.tile([1, 1], dt.float32, tag="lsum")
            nc.vector.tensor_reduce(out=lsum, in_=hdiff, op=ALU.add,
                                    axis=mybir.AxisListType.XY)
            nc.sync.dma_start(out=loss_d[:, :], in_=lsum)

    _split_waits(nc)
    return nc


# ---------------------------------------------------------------- entry point

def kernel(triple_emb, W_sfa, W1, b1, W2, b2, tri2path_size):
    _patch_tile_drain()
    triple_emb = np.asarray(triple_emb, np.float32)
    sizes_flat = np.asarray(tri2path_size, np.int32).reshape(-1).astype(np.int64)
    offsets = np.concatenate([[0], np.cumsum(sizes_flat)[:-1]])

    core_rows, bins_all, halves_all = _pack(sizes_flat)
    NB = max(len(b) for b in bins_all)
    NB = ((NB + 3) // 4) * 4
    NP0 = max(h for h in halves_all) // 2
    # all cores must share one program: normalize each core's half boundary
    # by padding half0 with empty bins up to 2*NP0
    for c in range(NCORES):
        h0 = halves_all[c]
        if h0 < 2 * NP0:
            bins_all[c] = (bins_all[c][:h0] + [[]] * (2 * NP0 - h0)
                           + bins_all[c][h0:])
    NB = max(max(len(b) for b in bins_all), NB)
    NB = ((NB + 3) // 4) * 4

    b1_np = np.asarray(b1, np.float32)
    with_b1 = bool(np.any(b1_np != 0.0))

    triple_bf = triple_emb.astype(bf16)
    wsfa_t = np.ascontiguousarray(
        np.asarray(W_sfa, np.float32).T.reshape(KC, 128, D).transpose(1, 0, 2)
        .reshape(128, KC * D)).astype(fp8e4)
    w1_t = np.ascontiguousarray(
        (np.asarray(W1, np.float32) * 8.0).T.reshape(KC, 128, 4 * D)
        .transpose(1, 0, 2).reshape(128, KC * 4 * D)).astype(fp8e4)
    w2_t = np.ascontiguousarray(
        np.repeat((np.asarray(W2, np.float32) * 16.0).reshape(HC, 128).T
                  [:, :, None], 16, axis=2).reshape(128, HC * 16)).astype(fp8e4)
    b1_r = b1_np.reshape(1, HC * 128).astype(bf16)
    b2_r = np.asarray(b2, np.float32).reshape(1, 1)
    in_maps = []
    for c in range(NCORES):
        xm, xt, slot_st = _build_core_arrays(bins_all[c], triple_bf, offsets, NB)
        m = {
            "x_bins": xm, "xt_bins": xt, "slot_of": slot_st,
            "w_sfa_t": wsfa_t, "w1_t": w1_t, "w2_t": w2_t,
            "b2_r": b2_r,
        }
        if with_b1:
            m["b1_r"] = b1_r
        in_maps.append(m)

    with _compile_lock:
        key = (NB, NP0, with_b1)
        nc = _compile_cache.get(key)
        if nc is None:
            nc = _build_program(NB, NP0, with_b1)
            _compile_cache[key] = nc

    res = run_bass_kernel_spmd(nc, in_maps, core_ids=list(range(NCORES)),
                               trace=bool(int(os.environ.get("KGE_TRACE", "0"))))
    total = np.float64(0.0)
    for r in res.results:
        total += np.float64(r["loss"][0, 0])
    total -= np.float64(NCORES * ROWS_PER_CORE * GAMMA)
    kernel.last_results = res
    return np.asarray(np.float32(total))
